# revision 1
# baseline (speedup 1.0000x reference)
"""Trainium2 Bass kernel for nn_Expansion (e3nn-style tensor-product expansion).

Math reformulation (verified against the jax reference):
  h   = silu(node_emb @ lw1 + lb1)                         [B,64]
  hb  = silu(node_emb @ bw1 + bb1)                         [B,64]
  x0  = feat[:,:128] @ W0 / sqrt(128)                      [B,16]
  x1k = feat[:,128+k::3] @ W1 / 8          (k=0,1,2)       [B,16]

The per-sample path contractions with wpath = (h @ lw2 + lb2) are a batched
bilinear form

  r[b,p] = sum_{c,w} h'[b,c] x[b,w] M[(c,w), p],   h' = [h, 1]

computed as a plain matmul over the outer product z[b,(c,w)] = h'[b,c]*x[b,w]
(K = 64*16 = 1024 in 8 chunks of 128, + a K=65 chunk for the bias MLP @ BB)
against reshaped weight matrices M built from lw2/bw2 on the host.  This
avoids materializing w = h@lw2 ([B,36864], ~600 MB) entirely.

v2 layout decisions (all driven by the instruction-cost timeline model):
  - The partition-replicated tiles hbc[q][(c8,w),b] = h[8q+c8,b] and
    xbc[t][(c8,w),b] = x_t[w,b] are produced DIRECTLY by the MLP/projection
    matmuls using host-replicated weight columns (lw1rep/W0rep/W1rep), so no
    separate selection-matrix matmuls or extra copies exist.  ACT applies
    silu (with replicated bias) straight from PSUM into bf16 SBUF tiles.
  - All matmuls run in bf16 (fp32 matmuls cost 4x); inputs ship as bf16.
  - The device output is the raw concatenation of the 9 PSUM bank groups per
    sample, [BC, 4352] bf16 — a fixed column permutation of the nonzero
    entries of the [80,80] block matrix.  The host scatters it into the
    final [B,80,80] float32 (incl. the blk11 diagonal triplication and the
    structural zeros), so the device does zero strided/duplicated writes.
  - First b-tile interleaves its three z0-consuming bank groups q-major so
    the PE consumption rate (3 matmuls / z0 chunk) matches ACT's silu
    production rate during warmup.

Sharding: pure data parallel, batch 4096 -> 8 cores x 512.  Weights
replicated; no cross-device communication.
"""

import sys

import numpy as np

sys.path.insert(0, "/opt/trn_rl_repo")

import ml_dtypes  # noqa: E402

B_TOTAL = 4096
N_CORES = 8
BC = B_TOTAL // N_CORES  # 512 samples per core
P = 128
NB = BC // P  # 4 b-tiles per core
C3 = 1.0 / np.sqrt(3.0)
NCOL = 4352  # packed device output columns per sample

# matmul dtype mode: "bf16" | "f32"
MM_MODE = "bf16"

_CACHE = {}


def _np_mm_dtype(mode):
    return ml_dtypes.bfloat16 if mode == "bf16" else np.float32


def _build_program(mode, skip_lb2):
    import concourse.tile as tile
    from concourse import bacc, mybir

    F32 = mybir.dt.float32
    MM = mybir.dt.bfloat16 if mode == "bf16" else mybir.dt.float32
    AF = mybir.ActivationFunctionType

    nc = bacc.Bacc("TRN2", target_bir_lowering=False, debug=False,
                   num_devices=N_CORES)

    t = {}
    t["featT"] = nc.dram_tensor("featT", [320, BC], MM, kind="ExternalInput").ap()
    t["wpk"] = nc.dram_tensor("wpk", [P, 1872], MM, kind="ExternalInput").ap()
    t["BBp"] = nc.dram_tensor("BBp", [65, 1280], MM, kind="ExternalInput").ap()
    t["R0"] = nc.dram_tensor("R0", [1024, 1280], MM, kind="ExternalInput").ap()
    t["R1"] = nc.dram_tensor("R1", [1024, 1024], MM, kind="ExternalInput").ap()
    if not skip_lb2:
        t["R0x"] = nc.dram_tensor("R0x", [16, 1280], MM, kind="ExternalInput").ap()
        t["R1x"] = nc.dram_tensor("R1x", [16, 1024], MM, kind="ExternalInput").ap()
    t["out"] = nc.dram_tensor("out", [BC, NCOL], MM, kind="ExternalOutput").ap()

    with tile.TileContext(nc) as tc:
        _emit(tc, t, mode, skip_lb2, mybir, MM, F32, AF)

    nc.compile()
    return nc


def _emit(tc, t, mode, skip_lb2, mybir, MM, F32, AF):
    nc = tc.nc
    from contextlib import ExitStack

    with ExitStack() as ctx:
        wpool = ctx.enter_context(tc.tile_pool(name="weights", bufs=1))
        apool = ctx.enter_context(tc.tile_pool(name="acts", bufs=1))
        zpool = ctx.enter_context(tc.tile_pool(name="z", bufs=1))
        opool = ctx.enter_context(tc.tile_pool(name="outs", bufs=3))
        prep_psum = ctx.enter_context(tc.tile_pool(name="prep_psum", bufs=3, space="PSUM"))
        main_psum = ctx.enter_context(tc.tile_pool(name="main_psum", bufs=5, space="PSUM"))

        # ---- SBUF tiles ----
        # wpk column layout: [embT(512) | lw1rep q0 | q1 | lb1rep(8) |
        #   bb1(1) | pad(7) | bw1(64) | W0rep(128) || lw1rep q2..q7 |
        #   W1rep(128, rows 0:64)]
        # The prefix [0:976] is everything the first prep matmuls need.
        wpk_sb = wpool.tile([P, 1872], MM, tag="wpk")
        BB_sb = wpool.tile([65, 1280], MM, tag="BBp")
        R0_sb = wpool.tile([P, 8, 1280], MM, tag="R0")
        R1_sb = wpool.tile([P, 8, 1024], MM, tag="R1")
        if not skip_lb2:
            R0x_sb = wpool.tile([16, 1280], MM, tag="R0x")
            R1x_sb = wpool.tile([16, 1024], MM, tag="R1x")

        feats_sb = apool.tile([P, BC], MM, tag="feats")
        featv_sb = apool.tile([64, 3, BC], MM, tag="featv")
        hbp_sb = apool.tile([65, BC], MM, tag="hbp")
        hbc = [apool.tile([P, BC], MM, name=f"hbc{q}", tag=f"hbc{q}")
               for q in range(8)]
        xbc = [apool.tile([P, BC], MM, name=f"xbc{t_}", tag=f"xbc{t_}")
               for t_ in range(4)]

        # ---- input DMAs, ordered by first consumer ----
        # R0 arrives q-chunk-major to match the phase-1 q-major consumption;
        # R1 follows in quarters (phase 2/3 consume it much later)
        r0v = t["R0"].rearrange("(q p) n -> p q n", p=P)
        r1v = t["R1"].rearrange("(q p) n -> p q n", p=P)
        nc.sync.dma_start(wpk_sb[:, 0:976], t["wpk"][:, 0:976])
        nc.sync.dma_start(feats_sb[:], t["featT"][0:128])
        nc.sync.dma_start(wpk_sb[:, 976:1872], t["wpk"][:, 976:1872])
        nc.sync.dma_start(R0_sb[:, 0, :], r0v[:, 0, :])
        nc.sync.dma_start(R0_sb[:, 1, :], r0v[:, 1, :])
        nc.sync.dma_start(R0_sb[:, 2, :], r0v[:, 2, :])
        nc.sync.dma_start(R0_sb[:, 3, :], r0v[:, 3, :])
        nc.sync.dma_start(BB_sb[:], t["BBp"][:])
        nc.sync.dma_start(R0_sb[:, 4, :], r0v[:, 4, :])
        nc.sync.dma_start(featv_sb[:],
                          t["featT"][128:320].rearrange("(k p) b -> p k b", k=3))
        for q in range(5, 8):
            nc.sync.dma_start(R0_sb[:, q, :], r0v[:, q, :])
        nc.sync.dma_start(R1_sb[:, 0:4, 0:512], r1v[:, 0:4, 0:512])
        nc.sync.dma_start(R1_sb[:, 4:8, 0:512], r1v[:, 4:8, 0:512])
        nc.sync.dma_start(R1_sb[:, 0:4, 512:1024], r1v[:, 0:4, 512:1024])
        nc.sync.dma_start(R1_sb[:, 4:8, 512:1024], r1v[:, 4:8, 512:1024])
        if not skip_lb2:
            nc.sync.dma_start(R0x_sb[:], t["R0x"][:])
            nc.sync.dma_start(R1x_sb[:], t["R1x"][:])

        # ---- PE warmup: dummy matmuls on a zeroed tile keep the PE busy
        # (and its p-state ramping) while the first input DMAs land; results
        # go to a scratch PSUM bank that is overwritten before any real use.
        warm_sb = apool.tile([P, P], MM, tag="warm")
        nc.vector.memset(warm_sb[:], 0.0)
        # preload the ACT activation table so the first real silu doesn't pay
        # the ~1.3us table-load latency (separate tile: no dep on warm_sb)
        tbl_sb = apool.tile([1, 4], MM, tag="tbl")
        nc.vector.memset(tbl_sb[:], 0.0)
        nc.scalar.activation(tbl_sb[0:1, 0:1], tbl_sb[0:1, 2:3], AF.Silu)
        pwarm = prep_psum.tile([P, P], F32, tag="pp")
        for _ in range(20):
            nc.tensor.matmul(pwarm[:], lhsT=warm_sb[:], rhs=warm_sb[:],
                             start=True, stop=True)

        # ---- prep emitters ----
        z = [[None] * 8 for _ in range(4)]
        HCOL = [512, 640, 976, 1104, 1232, 1360, 1488, 1616]

        def h_mm(q):
            # hbc[q][(c8,w),b] = silu((lw1rep_q)^T emb + lb1rep_q) = h[8q+c8,b]
            phq = prep_psum.tile([P, BC], F32, name=f"ph{q}", tag="pp")
            nc.tensor.matmul(phq[:], lhsT=wpk_sb[:, HCOL[q]:HCOL[q] + P],
                             rhs=wpk_sb[:, 0:512], start=True, stop=True)
            nc.scalar.activation(hbc[q][:], phq[:], AF.Silu,
                                 bias=wpk_sb[:, 768 + q:769 + q])

        def x_mm(tdx):
            # xbc[t][(c8,w),b] = x_t[w,b] via column-replicated W0/W1
            # (copy on DVE — ACT is saturated by the silu chain during prep)
            pxt = prep_psum.tile([P, BC], F32, name=f"px{tdx}", tag="pp")
            if tdx == 0:
                nc.tensor.matmul(pxt[:], lhsT=wpk_sb[:, 848:976],
                                 rhs=feats_sb[:], start=True, stop=True)
            else:
                nc.tensor.matmul(pxt[:], lhsT=wpk_sb[0:64, 1744:1872],
                                 rhs=featv_sb[:, tdx - 1, :],
                                 start=True, stop=True)
            nc.vector.tensor_copy(out=xbc[tdx][:], in_=pxt[:])

        def z_mul(tdx, q):
            zt = zpool.tile([P, BC], MM, name=f"z{tdx}_{q}", tag=f"z{tdx}_{q}")
            nc.vector.tensor_mul(out=zt[:], in0=hbc[q][:], in1=xbc[tdx][:])
            z[tdx][q] = zt

        h_mm(0)
        x_mm(0)
        h_mm(1)

        # hbp[c,b] = silu(bw1^T emb + bb1), plus a ones row for the bb2 path
        ph = prep_psum.tile([64, BC], F32, tag="pp")
        nc.tensor.matmul(ph[:], lhsT=wpk_sb[:, 784:848], rhs=wpk_sb[:, 0:512],
                         start=True, stop=True)
        nc.scalar.activation(hbp_sb[0:64, :], ph[:], AF.Silu,
                             bias=wpk_sb[0:64, 776:777])
        nc.gpsimd.memset(hbp_sb[64:65, :], 1.0)

        # fillers: cover the silu->z_mul latency before the first z matmul
        for _ in range(10):
            nc.tensor.matmul(pwarm[:], lhsT=warm_sb[:], rhs=warm_sb[:],
                             start=True, stop=True)

        # ---- main accumulation groups, group-type-major ----
        # phase 1 (needs only R0): per b-tile g0..g2 = z0 @ R0 cols
        # (0:512 | 512:1024 | 1024:1280) + the BB bias chunk, q-major
        # interleaved across the three banks.  The j==0 pass interleaves the
        # remaining prep matmuls so the PE tracks ACT's silu cadence.
        # phase 2 (R1[:, 0:512]): g3..g5 = z[1+k] @ R1 left  (blk01)
        # phase 3 (R1[:, 512:1024]): g6..g8 = z[1+i] @ R1 right (blk10)
        def copy_out(eng, dst_ap, src_ap):
            if eng == "a":
                nc.scalar.copy(dst_ap, src_ap)
            else:
                nc.vector.tensor_copy(out=dst_ap, in_=src_ap)

        out_t = [opool.tile([P, NCOL], MM, name=f"out_t{j}", tag=f"out_t{j}")
                 for j in range(NB)]
        P1_ENG = ['aaa', 'aaa', 'aaa', 'add']  # per-j engines for g0/g1/g2
        P23_ENG = ['adadad', 'dadada', 'adadad', 'dadada']  # per-j g3..g8

        def phase1(j, final):
            bsl = slice(P * j, P * (j + 1))
            gcols = [(0, 512), (512, 1024), (1024, 1280)]
            if final:
                # sequential groups with per-group writeback: only the last
                # (smallest) group's copy+DMA trail the final matmul
                for g in range(3):
                    c0, c1 = gcols[g]
                    pg = main_psum.tile([P, 512], F32, name=f"pg{j}_{g}",
                                        tag="mp")
                    psl = pg[:, 0:c1 - c0]
                    for q in range(8):
                        nc.tensor.matmul(psl, lhsT=z[0][q][:, bsl],
                                         rhs=R0_sb[:, q, c0:c1],
                                         start=(q == 0), stop=False)
                    if not skip_lb2:
                        nc.tensor.matmul(psl, lhsT=xbc[0][0:16, bsl],
                                         rhs=R0x_sb[:, c0:c1],
                                         start=False, stop=False)
                    nc.tensor.matmul(psl, lhsT=hbp_sb[:, bsl],
                                     rhs=BB_sb[:, c0:c1],
                                     start=False, stop=True)
                    copy_out(P1_ENG[j][g], out_t[j][:, c0:c1], psl)
                    nc.sync.dma_start(t["out"][bsl, c0:c1],
                                      out_t[j][:, c0:c1])
                return
            pg = [main_psum.tile([P, 512], F32, name=f"pg{j}_{g}", tag="mp")
                  for g in range(3)]
            gsl = [pg[0][:], pg[1][:], pg[2][:, 0:256]]
            for q in range(8):
                if j == 0:
                    z_mul(0, q)
                for g in range(3):
                    c0, c1 = gcols[g]
                    nc.tensor.matmul(gsl[g], lhsT=z[0][q][:, bsl],
                                     rhs=R0_sb[:, q, c0:c1],
                                     start=(q == 0), stop=False)
                if j == 0:
                    # the next replication matmul comes AFTER this q's main
                    # matmuls so a late weight DMA can't block them in-order
                    if q < 6:
                        h_mm(q + 2)
                    else:
                        x_mm(1 if q == 6 else 2)
            if j == 0:
                x_mm(3)
                for tdx in range(1, 4):
                    for q in range(8):
                        z_mul(tdx, q)
            if not skip_lb2:
                for g in range(3):
                    c0, c1 = gcols[g]
                    nc.tensor.matmul(gsl[g], lhsT=xbc[0][0:16, bsl],
                                     rhs=R0x_sb[:, c0:c1],
                                     start=False, stop=False)
            for g in range(3):
                c0, c1 = gcols[g]
                nc.tensor.matmul(gsl[g], lhsT=hbp_sb[:, bsl],
                                 rhs=BB_sb[:, c0:c1], start=False, stop=True)
            for g in range(3):
                c0, c1 = gcols[g]
                copy_out(P1_ENG[j][g], out_t[j][:, c0:c1], gsl[g])
            nc.sync.dma_start(t["out"][bsl, 0:1280], out_t[j][:, 0:1280])

        for j in range(NB - 1):
            phase1(j, final=False)

        for phase in range(2):
            rc = (0, 512) if phase == 0 else (512, 1024)
            for j in range(NB):
                bsl = slice(P * j, P * (j + 1))
                for mi in range(3):
                    m = 3 * phase + mi
                    tdx = 1 + mi
                    pgm = main_psum.tile([P, 512], F32, name=f"pm{j}_{m}",
                                         tag="mp")
                    for q in range(8):
                        last = skip_lb2 and q == 7
                        nc.tensor.matmul(pgm[:], lhsT=z[tdx][q][:, bsl],
                                         rhs=R1_sb[:, q, rc[0]:rc[1]],
                                         start=(q == 0), stop=last)
                    if not skip_lb2:
                        nc.tensor.matmul(pgm[:], lhsT=xbc[tdx][0:16, bsl],
                                         rhs=R1x_sb[:, rc[0]:rc[1]],
                                         start=False, stop=True)
                    c0 = 1280 + 512 * m
                    copy_out(P23_ENG[j][m], out_t[j][:, c0:c0 + 512], pgm[:])
                c0 = 1280 + 1536 * phase
                nc.sync.dma_start(t["out"][bsl, c0:c0 + 1536],
                                  out_t[j][:, c0:c0 + 1536])

        phase1(NB - 1, final=True)


def _prepare(inputs, mode):
    f32 = np.float32
    feat = np.ascontiguousarray(np.asarray(inputs["feat"], dtype=f32))
    node_emb = np.ascontiguousarray(np.asarray(inputs["node_emb"], dtype=f32))
    W0 = np.asarray(inputs["W0"], f32)
    W1 = np.asarray(inputs["W1"], f32)
    lw1 = np.asarray(inputs["lw1"], f32)
    lb1 = np.asarray(inputs["lb1"], f32)
    lw2 = np.asarray(inputs["lw2"], f32)
    lb2 = np.asarray(inputs["lb2"], f32)
    bw1 = np.asarray(inputs["bw1"], f32)
    bb1 = np.asarray(inputs["bb1"], f32)
    bw2 = np.asarray(inputs["bw2"], f32)
    bb2 = np.asarray(inputs["bb2"], f32)

    mmnp = _np_mm_dtype(mode)
    s16 = np.float32(1.0 / 16.0)
    sC = np.float32(C3 / 16.0)

    # weight matrices for the main contraction, path scales folded in
    lw2p = np.concatenate([lw2, lb2[None]], axis=0)           # [65, 36864]
    M00 = lw2p[:, :16384].reshape(1040, 1024) * s16
    M11 = lw2p[:, 16384:20480].reshape(1040, 256) * sC
    M01 = lw2p[:, 20480:28672].reshape(1040, 512) * sC
    M10 = lw2p[:, 28672:36864].reshape(1040, 512) * sC
    R0f = np.concatenate([M00, M11], axis=1)                  # [1040, 1280]
    R1f = np.concatenate([M01, M10], axis=1)                  # [1040, 1024]
    R0 = np.ascontiguousarray(R0f[0:1024]).astype(mmnp)
    R1 = np.ascontiguousarray(R1f[0:1024]).astype(mmnp)
    R0x = np.ascontiguousarray(R0f[1024:1040]).astype(mmnp)
    R1x = np.ascontiguousarray(R1f[1024:1040]).astype(mmnp)
    BBf = np.concatenate([bw2, bb2[None]], axis=0)            # [65, 1280]
    BBp = np.ascontiguousarray(
        np.concatenate([BBf[:, :1024] * s16, BBf[:, 1024:] * sC], axis=1)
    ).astype(mmnp)

    # replicated-column weights: output partition (c8,w) = 16*c8 + w
    W0s = W0 * np.float32(1.0 / np.sqrt(128.0))               # [128, 16]
    W1s = W1 * np.float32(1.0 / 8.0)                          # [64, 16]
    rep = np.arange(1024)
    gsel = (rep // 128) * 8 + (rep % 128) // 16               # c = 8q + c8
    lw1rep = lw1[:, gsel]                                     # [128, 1024]
    W0rep = np.tile(W0s, (1, 8))                              # [128, 128]
    W1rep = np.tile(W1s, (1, 8))                              # [64, 128]
    # layout must match HCOL & friends in _emit (embT in cols 0:512,
    # filled per core below)
    wpk = np.zeros((128, 1872), f32)
    hcol = [512, 640, 976, 1104, 1232, 1360, 1488, 1616]
    for q in range(8):
        wpk[:, hcol[q]:hcol[q] + 128] = lw1rep[:, 128 * q:128 * (q + 1)]
        wpk[:, 768 + q] = lb1[8 * q + np.arange(128) // 16]
    wpk[0:64, 776] = bb1
    wpk[:, 784:848] = bw1
    wpk[:, 848:976] = W0rep
    wpk[0:64, 1744:1872] = W1rep
    wpk = wpk.astype(mmnp)

    skip_lb2 = not bool(np.any(lb2))

    in_maps = []
    for i in range(N_CORES):
        sl = slice(i * BC, (i + 1) * BC)
        fs = feat[sl]
        featT = np.ascontiguousarray(
            np.concatenate(
                [fs[:, :128], fs[:, 128::3], fs[:, 129::3], fs[:, 130::3]],
                axis=1).T.astype(mmnp))                       # [320, BC]
        wpk_i = wpk.copy()
        wpk_i[:, 0:512] = node_emb[sl].T.astype(mmnp)
        m = {
            "featT": featT,
            "wpk": np.ascontiguousarray(wpk_i), "BBp": BBp,
            "R0": R0, "R1": R1,
        }
        if not skip_lb2:
            m["R0x"] = R0x
            m["R1x"] = R1x
        in_maps.append(m)
    return in_maps, skip_lb2


def _unpack_output(buf):
    """[B, 4352] packed columns -> [B, 80, 80] float32."""
    bf = buf.astype(np.float32)
    n = bf.shape[0]
    out3 = np.zeros((n, 80, 80), np.float32)
    out3[:, 0:16, 0:32] = bf[:, 0:512].reshape(n, 16, 32)
    out3[:, 16:32, 0:32] = bf[:, 512:1024].reshape(n, 16, 32)
    p11 = bf[:, 1024:1280].reshape(n, 16, 16)
    for i in range(3):
        out3[:, 32 + i::3, 32 + i::3] = p11
    for k in range(3):
        out3[:, 0:32, 32 + k::3] = \
            bf[:, 1280 + 512 * k:1792 + 512 * k].reshape(n, 32, 16)
    for i in range(3):
        out3[:, 32 + i::3, 0:32] = \
            bf[:, 2816 + 512 * i:3328 + 512 * i].reshape(n, 16, 32)
    return out3


def run(inputs, mode=None, trace=False):
    """Build (cached), run on 8 cores, gather. Returns (out, results)."""
    mode = mode or MM_MODE
    in_maps, skip_lb2 = _prepare(inputs, mode)
    key = (mode, skip_lb2)
    if key not in _CACHE:
        _CACHE[key] = _build_program(mode, skip_lb2)
    nc = _CACHE[key]

    from concourse.bass_utils import run_bass_kernel_spmd
    for attempt in range(3):
        res = run_bass_kernel_spmd(nc, in_maps, list(range(N_CORES)),
                                   trace=trace)
        buf = np.concatenate([res.results[i]["out"] for i in range(N_CORES)],
                             axis=0)
        out = _unpack_output(buf)
        # guard against a rare transport/device flake observed to return NaN
        # payloads; correct runs are deterministic, so retrying is safe
        if not np.isnan(out).any():
            return out, res
    return out, res


def kernel(**inputs):
    out, _ = run(inputs)
    return out



# revision 16
# speedup vs baseline: 1.3687x; 1.3687x over previous
"""Trainium2 Bass kernel for nn_Expansion (e3nn-style tensor-product expansion).

Math reformulation (verified against the jax reference):
  h   = silu(node_emb @ lw1 + lb1)                         [B,64]
  hb  = silu(node_emb @ bw1 + bb1)                         [B,64]
  x0  = feat[:,:128] @ W0 / sqrt(128)                      [B,16]
  x1k = feat[:,128+k::3] @ W1 / 8          (k=0,1,2)       [B,16]

The per-sample path contractions with wpath = (h @ lw2 + lb2) are a batched
bilinear form

  r[b,p] = sum_{c,w} h'[b,c] x[b,w] M[(c,w), p],   h' = [h, 1]

computed as a plain matmul over the outer product z[b,(c,w)] = h'[b,c]*x[b,w]
(K = 64*16 = 1024 in 8 chunks of 128, + a K=65 chunk for the bias MLP @ BB)
against reshaped weight matrices M built from lw2/bw2 on the host.  This
avoids materializing w = h@lw2 ([B,36864], ~600 MB) entirely.

v2 layout decisions (all driven by the instruction-cost timeline model):
  - The partition-replicated tiles hbc[q][(c8,w),b] = h[8q+c8,b] and
    xbc[t][(c8,w),b] = x_t[w,b] are produced DIRECTLY by the MLP/projection
    matmuls using host-replicated weight columns (lw1rep/W0rep/W1rep), so no
    separate selection-matrix matmuls or extra copies exist.  ACT applies
    silu (with replicated bias) straight from PSUM into bf16 SBUF tiles.
  - All matmuls run in bf16 (fp32 matmuls cost 4x); inputs ship as bf16.
  - The device output is the raw concatenation of the 9 PSUM bank groups per
    sample, [BC, 4352] bf16 — a fixed column permutation of the nonzero
    entries of the [80,80] block matrix.  The host scatters it into the
    final [B,80,80] float32 (incl. the blk11 diagonal triplication and the
    structural zeros), so the device does zero strided/duplicated writes.
  - First b-tile interleaves its three z0-consuming bank groups q-major so
    the PE consumption rate (3 matmuls / z0 chunk) matches ACT's silu
    production rate during warmup.

Sharding: pure data parallel, batch 4096 -> 8 cores x 512.  Weights
replicated; no cross-device communication.

v3: phases 2/3 (blk01/blk10, 69% of the matmul rows) run as fp8e4
DoubleRow matmuls.  Each pair of K-chunks (2q, 2q+1) becomes two
K=256 DoubleRow matmuls -- one against an e4m3 R1_hi pair, one against
the e4m3 residual R1_lo pair -- at 0.5 cycles/row each, halving the
phase-2/3 PE time.  The R side is exact to ~6e-4 (hi+lo split, scaled
x4096 to clear the e4m3 denormal floor; the 2^-12 descale is folded
into the PSUM->SBUF copies).  The z side is a single e4m3 rounding
(sigma ~2.6e-2) applied only to the blk01/blk10 paths (42.6% of output
norm^2), leaving total rel err ~1.8e-2 vs the 2e-2 gate (verified
bit-exact RNE conversion on device).  blk00/blk11 (phase 1) stay bf16.
"""

import sys

import numpy as np

sys.path.insert(0, "/opt/trn_rl_repo")

import ml_dtypes  # noqa: E402

B_TOTAL = 4096
N_CORES = 8
BC = B_TOTAL // N_CORES  # 512 samples per core
P = 128
NB = BC // P  # 4 b-tiles per core
C3 = 1.0 / np.sqrt(3.0)
NCOL = 4352  # packed device output columns per sample

# matmul dtype mode: "bf16" | "f32"
MM_MODE = "bf16"
# fp8e4 DoubleRow for phases 2/3 (blk01/blk10); only in bf16 mode
FP8_P23 = True
R1_SCALE = 4096.0  # lifts e4m3(R1) out of the denormal floor
R1_DESCALE = 1.0 / R1_SCALE

_CACHE = {}


def _np_mm_dtype(mode):
    return ml_dtypes.bfloat16 if mode == "bf16" else np.float32


def _build_program(mode, skip_lb2, fp8):
    import concourse.tile as tile
    from concourse import bacc, mybir

    F32 = mybir.dt.float32
    MM = mybir.dt.bfloat16 if mode == "bf16" else mybir.dt.float32
    AF = mybir.ActivationFunctionType

    nc = bacc.Bacc("TRN2", target_bir_lowering=False, debug=False,
                   num_devices=N_CORES)

    t = {}
    t["featT"] = nc.dram_tensor("featT", [320, BC], MM, kind="ExternalInput").ap()
    t["wpk"] = nc.dram_tensor("wpk", [P, 1872], MM, kind="ExternalInput").ap()
    t["BBp"] = nc.dram_tensor("BBp", [65, 1280], MM, kind="ExternalInput").ap()
    t["R0"] = nc.dram_tensor("R0", [1024, 1280], MM, kind="ExternalInput").ap()
    if fp8:
        FP8 = mybir.dt.float8e4
        # [p, col-half, qq, subtile, n'] -- hi and e4m3 residual pairs
        t["R1h"] = nc.dram_tensor("R1h", [P, 2, 4, 2, 512], FP8,
                                  kind="ExternalInput").ap()
        t["R1l"] = nc.dram_tensor("R1l", [P, 2, 4, 2, 512], FP8,
                                  kind="ExternalInput").ap()
    else:
        t["R1"] = nc.dram_tensor("R1", [1024, 1024], MM, kind="ExternalInput").ap()
    if not skip_lb2:
        t["R0x"] = nc.dram_tensor("R0x", [16, 1280], MM, kind="ExternalInput").ap()
        t["R1x"] = nc.dram_tensor("R1x", [16, 1024], MM, kind="ExternalInput").ap()
    t["out"] = nc.dram_tensor("out", [BC, NCOL], MM, kind="ExternalOutput").ap()

    with tile.TileContext(nc) as tc:
        _emit(tc, t, mode, skip_lb2, fp8, mybir, MM, F32, AF)

    nc.compile()
    return nc


def _emit(tc, t, mode, skip_lb2, fp8, mybir, MM, F32, AF):
    nc = tc.nc
    FP8 = mybir.dt.float8e4
    from contextlib import ExitStack

    with ExitStack() as ctx:
        wpool = ctx.enter_context(tc.tile_pool(name="weights", bufs=1))
        apool = ctx.enter_context(tc.tile_pool(name="acts", bufs=1))
        zpool = ctx.enter_context(tc.tile_pool(name="z", bufs=1))
        opool = ctx.enter_context(tc.tile_pool(name="outs", bufs=3))
        prep_psum = ctx.enter_context(tc.tile_pool(name="prep_psum", bufs=3, space="PSUM"))
        main_psum = ctx.enter_context(tc.tile_pool(name="main_psum", bufs=5, space="PSUM"))

        # ---- SBUF tiles ----
        # wpk column layout: [embT(512) | lw1rep q0 | q1 | lb1rep(8) |
        #   bb1(1) | pad(7) | bw1(64) | W0rep(128) || lw1rep q2..q7 |
        #   W1rep(128, rows 0:64)]
        # The prefix [0:976] is everything the first prep matmuls need.
        wpk_sb = wpool.tile([P, 1872], MM, tag="wpk")
        BB_sb = wpool.tile([65, 1280], MM, tag="BBp")
        R0_sb = wpool.tile([P, 8, 1280], MM, tag="R0")
        if fp8:
            R1h_sb = wpool.tile([P, 2, 4, 2, 512], FP8, tag="R1h")
            R1l_sb = wpool.tile([P, 2, 4, 2, 512], FP8, tag="R1l")
        else:
            R1_sb = wpool.tile([P, 8, 1024], MM, tag="R1")
        if not skip_lb2:
            R0x_sb = wpool.tile([16, 1280], MM, tag="R0x")
            R1x_sb = wpool.tile([16, 1024], MM, tag="R1x")

        feats_sb = apool.tile([P, BC], MM, tag="feats")
        featv_sb = apool.tile([64, 3, BC], MM, tag="featv")
        hbp_sb = apool.tile([65, BC], MM, tag="hbp")
        hbc = [apool.tile([P, BC], MM, name=f"hbc{q}", tag=f"hbc{q}")
               for q in range(8)]
        xbc = [apool.tile([P, BC], MM, name=f"xbc{t_}", tag=f"xbc{t_}")
               for t_ in range(4)]

        # ---- input DMAs, ordered by first consumer ----
        # R0 arrives q-chunk-major to match the phase-1 q-major consumption;
        # R1 follows in halves (phase 2/3 consume it much later)
        r0v = t["R0"].rearrange("(q p) n -> p q n", p=P)
        nc.sync.dma_start(wpk_sb[:, 0:976], t["wpk"][:, 0:976])
        nc.sync.dma_start(feats_sb[:], t["featT"][0:128])
        nc.sync.dma_start(wpk_sb[:, 976:1872], t["wpk"][:, 976:1872])
        nc.sync.dma_start(R0_sb[:, 0, :], r0v[:, 0, :])
        nc.sync.dma_start(R0_sb[:, 1, :], r0v[:, 1, :])
        nc.sync.dma_start(R0_sb[:, 2, :], r0v[:, 2, :])
        nc.sync.dma_start(R0_sb[:, 3, :], r0v[:, 3, :])
        nc.sync.dma_start(BB_sb[:], t["BBp"][:])
        nc.sync.dma_start(R0_sb[:, 4, :], r0v[:, 4, :])
        nc.sync.dma_start(featv_sb[:],
                          t["featT"][128:320].rearrange("(k p) b -> p k b", k=3))
        for q in range(5, 8):
            nc.sync.dma_start(R0_sb[:, q, :], r0v[:, q, :])
        if fp8:
            nc.sync.dma_start(R1h_sb[:, 0], t["R1h"][:, 0])
            nc.sync.dma_start(R1l_sb[:, 0], t["R1l"][:, 0])
            nc.sync.dma_start(R1h_sb[:, 1], t["R1h"][:, 1])
            nc.sync.dma_start(R1l_sb[:, 1], t["R1l"][:, 1])
        else:
            r1v = t["R1"].rearrange("(q p) n -> p q n", p=P)
            nc.sync.dma_start(R1_sb[:, 0:4, 0:512], r1v[:, 0:4, 0:512])
            nc.sync.dma_start(R1_sb[:, 4:8, 0:512], r1v[:, 4:8, 0:512])
            nc.sync.dma_start(R1_sb[:, 0:4, 512:1024], r1v[:, 0:4, 512:1024])
            nc.sync.dma_start(R1_sb[:, 4:8, 512:1024], r1v[:, 4:8, 512:1024])
        if not skip_lb2:
            nc.sync.dma_start(R0x_sb[:], t["R0x"][:])
            nc.sync.dma_start(R1x_sb[:], t["R1x"][:])

        # ---- PE warmup: dummy matmuls on a zeroed tile keep the PE busy
        # (and its p-state ramping) while the first input DMAs land; results
        # go to a scratch PSUM bank that is overwritten before any real use.
        warm_sb = apool.tile([P, P], MM, tag="warm")
        nc.vector.memset(warm_sb[:], 0.0)
        # preload the ACT activation table so the first real silu doesn't pay
        # the ~1.3us table-load latency (separate tile: no dep on warm_sb)
        tbl_sb = apool.tile([1, 4], MM, tag="tbl")
        nc.vector.memset(tbl_sb[:], 0.0)
        nc.scalar.activation(tbl_sb[0:1, 0:1], tbl_sb[0:1, 2:3], AF.Silu)
        pwarm = prep_psum.tile([P, P], F32, tag="pp")
        for _ in range(20):
            nc.tensor.matmul(pwarm[:], lhsT=warm_sb[:], rhs=warm_sb[:],
                             start=True, stop=True)

        # ---- prep emitters ----
        z = [[None] * 8 for _ in range(4)]
        zp = [[None] * 4 for _ in range(4)]  # fp8 chunk-pair tiles, tdx>=1
        HCOL = [512, 640, 976, 1104, 1232, 1360, 1488, 1616]

        def h_mm(q):
            # hbc[q][(c8,w),b] = silu((lw1rep_q)^T emb + lb1rep_q) = h[8q+c8,b]
            phq = prep_psum.tile([P, BC], F32, name=f"ph{q}", tag="pp")
            nc.tensor.matmul(phq[:], lhsT=wpk_sb[:, HCOL[q]:HCOL[q] + P],
                             rhs=wpk_sb[:, 0:512], start=True, stop=True)
            nc.scalar.activation(hbc[q][:], phq[:], AF.Silu,
                                 bias=wpk_sb[:, 768 + q:769 + q])

        def x_mm(tdx):
            # xbc[t][(c8,w),b] = x_t[w,b] via column-replicated W0/W1
            # (copy on DVE — ACT is saturated by the silu chain during prep)
            pxt = prep_psum.tile([P, BC], F32, name=f"px{tdx}", tag="pp")
            if tdx == 0:
                nc.tensor.matmul(pxt[:], lhsT=wpk_sb[:, 848:976],
                                 rhs=feats_sb[:], start=True, stop=True)
            else:
                nc.tensor.matmul(pxt[:], lhsT=wpk_sb[0:64, 1744:1872],
                                 rhs=featv_sb[:, tdx - 1, :],
                                 start=True, stop=True)
            nc.vector.tensor_copy(out=xbc[tdx][:], in_=pxt[:])

        def z_mul(tdx, q):
            if fp8 and tdx >= 1:
                # e4m3 z, written into subtile slot q%2 of the (2q', 2q'+1)
                # DoubleRow pair tile (single RNE rounding off the bf16 mul)
                qq = q // 2
                if zp[tdx][qq] is None:
                    zp[tdx][qq] = zpool.tile([P, 2, BC], FP8,
                                             name=f"zp{tdx}_{qq}",
                                             tag=f"zp{tdx}_{qq}")
                nc.vector.tensor_mul(out=zp[tdx][qq][:, q % 2, :],
                                     in0=hbc[q][:], in1=xbc[tdx][:])
                return
            zt = zpool.tile([P, BC], MM, name=f"z{tdx}_{q}", tag=f"z{tdx}_{q}")
            nc.vector.tensor_mul(out=zt[:], in0=hbc[q][:], in1=xbc[tdx][:])
            z[tdx][q] = zt

        h_mm(0)
        x_mm(0)
        h_mm(1)

        # hbp[c,b] = silu(bw1^T emb + bb1), plus a ones row for the bb2 path
        ph = prep_psum.tile([64, BC], F32, tag="pp")
        nc.tensor.matmul(ph[:], lhsT=wpk_sb[:, 784:848], rhs=wpk_sb[:, 0:512],
                         start=True, stop=True)
        nc.scalar.activation(hbp_sb[0:64, :], ph[:], AF.Silu,
                             bias=wpk_sb[0:64, 776:777])
        nc.gpsimd.memset(hbp_sb[64:65, :], 1.0)

        # fillers: cover the silu->z_mul latency before the first z matmul
        for _ in range(10):
            nc.tensor.matmul(pwarm[:], lhsT=warm_sb[:], rhs=warm_sb[:],
                             start=True, stop=True)

        # ---- main accumulation groups, group-type-major ----
        # phase 1 (needs only R0): per b-tile g0..g2 = z0 @ R0 cols
        # (0:512 | 512:1024 | 1024:1280) + the BB bias chunk, q-major
        # interleaved across the three banks.  The j==0 pass interleaves the
        # remaining prep matmuls so the PE tracks ACT's silu cadence.
        # phase 2 (R1[:, 0:512]): g3..g5 = z[1+k] @ R1 left  (blk01)
        # phase 3 (R1[:, 512:1024]): g6..g8 = z[1+i] @ R1 right (blk10)
        def copy_out(eng, dst_ap, src_ap, scale=None):
            if eng == "a":
                if scale is None:
                    nc.scalar.copy(dst_ap, src_ap)
                else:
                    nc.scalar.mul(dst_ap, src_ap, scale)
            else:
                if scale is None:
                    nc.vector.tensor_copy(out=dst_ap, in_=src_ap)
                else:
                    nc.vector.tensor_scalar_mul(dst_ap, src_ap, scale)

        out_t = [opool.tile([P, NCOL], MM, name=f"out_t{j}", tag=f"out_t{j}")
                 for j in range(NB)]
        P1_ENG = ['aaa', 'aaa', 'aaa', 'add']  # per-j engines for g0/g1/g2
        P23_ENG = ['adadad', 'dadada', 'adadad', 'dadada']  # per-j g3..g8

        def phase1(j, final):
            bsl = slice(P * j, P * (j + 1))
            gcols = [(0, 512), (512, 1024), (1024, 1280)]
            if final:
                # sequential groups with per-group writeback: only the last
                # (smallest) group's copy+DMA trail the final matmul
                for g in range(3):
                    c0, c1 = gcols[g]
                    pg = main_psum.tile([P, 512], F32, name=f"pg{j}_{g}",
                                        tag="mp")
                    psl = pg[:, 0:c1 - c0]
                    for q in range(8):
                        nc.tensor.matmul(psl, lhsT=z[0][q][:, bsl],
                                         rhs=R0_sb[:, q, c0:c1],
                                         start=(q == 0), stop=False)
                    if not skip_lb2:
                        nc.tensor.matmul(psl, lhsT=xbc[0][0:16, bsl],
                                         rhs=R0x_sb[:, c0:c1],
                                         start=False, stop=False)
                    nc.tensor.matmul(psl, lhsT=hbp_sb[:, bsl],
                                     rhs=BB_sb[:, c0:c1],
                                     start=False, stop=True)
                    copy_out(P1_ENG[j][g], out_t[j][:, c0:c1], psl)
                    nc.sync.dma_start(t["out"][bsl, c0:c1],
                                      out_t[j][:, c0:c1])
                return
            pg = [main_psum.tile([P, 512], F32, name=f"pg{j}_{g}", tag="mp")
                  for g in range(3)]
            gsl = [pg[0][:], pg[1][:], pg[2][:, 0:256]]
            for q in range(8):
                if j == 0:
                    z_mul(0, q)
                for g in range(3):
                    c0, c1 = gcols[g]
                    nc.tensor.matmul(gsl[g], lhsT=z[0][q][:, bsl],
                                     rhs=R0_sb[:, q, c0:c1],
                                     start=(q == 0), stop=False)
                if j == 0:
                    # the next replication matmul comes AFTER this q's main
                    # matmuls so a late weight DMA can't block them in-order
                    if q < 6:
                        h_mm(q + 2)
                    else:
                        x_mm(1 if q == 6 else 2)
            if j == 0:
                x_mm(3)
                for tdx in range(1, 4):
                    for q in range(8):
                        z_mul(tdx, q)
            if not skip_lb2:
                for g in range(3):
                    c0, c1 = gcols[g]
                    nc.tensor.matmul(gsl[g], lhsT=xbc[0][0:16, bsl],
                                     rhs=R0x_sb[:, c0:c1],
                                     start=False, stop=False)
            for g in range(3):
                c0, c1 = gcols[g]
                nc.tensor.matmul(gsl[g], lhsT=hbp_sb[:, bsl],
                                 rhs=BB_sb[:, c0:c1], start=False, stop=True)
            for g in range(3):
                c0, c1 = gcols[g]
                copy_out(P1_ENG[j][g], out_t[j][:, c0:c1], gsl[g])
            nc.sync.dma_start(t["out"][bsl, 0:1280], out_t[j][:, 0:1280])

        for j in range(NB - 1):
            phase1(j, final=False)

        DRM = mybir.MatmulPerfMode.DoubleRow
        for phase in range(2):
            rc = (0, 512) if phase == 0 else (512, 1024)
            for j in range(NB):
                bsl = slice(P * j, P * (j + 1))
                for mi in range(3):
                    m = 3 * phase + mi
                    tdx = 1 + mi
                    pgm = main_psum.tile([P, 512], F32, name=f"pm{j}_{m}",
                                         tag="mp")
                    if fp8:
                        for qq in range(4):
                            nc.tensor.matmul(pgm[:],
                                             lhsT=zp[tdx][qq][:, :, bsl],
                                             rhs=R1h_sb[:, phase, qq, :, :],
                                             start=(qq == 0), stop=False,
                                             perf_mode=DRM)
                            nc.tensor.matmul(pgm[:],
                                             lhsT=zp[tdx][qq][:, :, bsl],
                                             rhs=R1l_sb[:, phase, qq, :, :],
                                             start=False,
                                             stop=(skip_lb2 and qq == 3),
                                             perf_mode=DRM)
                    else:
                        for q in range(8):
                            last = skip_lb2 and q == 7
                            nc.tensor.matmul(pgm[:], lhsT=z[tdx][q][:, bsl],
                                             rhs=R1_sb[:, q, rc[0]:rc[1]],
                                             start=(q == 0), stop=last)
                    if not skip_lb2:
                        nc.tensor.matmul(pgm[:], lhsT=xbc[tdx][0:16, bsl],
                                         rhs=R1x_sb[:, rc[0]:rc[1]],
                                         start=False, stop=True)
                    c0 = 1280 + 512 * m
                    copy_out(P23_ENG[j][m], out_t[j][:, c0:c0 + 512], pgm[:],
                             scale=(R1_DESCALE if fp8 else None))
                c0 = 1280 + 1536 * phase
                nc.sync.dma_start(t["out"][bsl, c0:c0 + 1536],
                                  out_t[j][:, c0:c0 + 1536])

        phase1(NB - 1, final=True)


def _prepare(inputs, mode):
    f32 = np.float32
    feat = np.ascontiguousarray(np.asarray(inputs["feat"], dtype=f32))
    node_emb = np.ascontiguousarray(np.asarray(inputs["node_emb"], dtype=f32))
    W0 = np.asarray(inputs["W0"], f32)
    W1 = np.asarray(inputs["W1"], f32)
    lw1 = np.asarray(inputs["lw1"], f32)
    lb1 = np.asarray(inputs["lb1"], f32)
    lw2 = np.asarray(inputs["lw2"], f32)
    lb2 = np.asarray(inputs["lb2"], f32)
    bw1 = np.asarray(inputs["bw1"], f32)
    bb1 = np.asarray(inputs["bb1"], f32)
    bw2 = np.asarray(inputs["bw2"], f32)
    bb2 = np.asarray(inputs["bb2"], f32)

    mmnp = _np_mm_dtype(mode)
    s16 = np.float32(1.0 / 16.0)
    sC = np.float32(C3 / 16.0)

    # weight matrices for the main contraction, path scales folded in
    lw2p = np.concatenate([lw2, lb2[None]], axis=0)           # [65, 36864]
    M00 = lw2p[:, :16384].reshape(1040, 1024) * s16
    M11 = lw2p[:, 16384:20480].reshape(1040, 256) * sC
    M01 = lw2p[:, 20480:28672].reshape(1040, 512) * sC
    M10 = lw2p[:, 28672:36864].reshape(1040, 512) * sC
    R0f = np.concatenate([M00, M11], axis=1)                  # [1040, 1280]
    R1f = np.concatenate([M01, M10], axis=1)                  # [1040, 1024]
    fp8 = FP8_P23 and mode == "bf16"
    R0 = np.ascontiguousarray(R0f[0:1024]).astype(mmnp)
    R0x = np.ascontiguousarray(R0f[1024:1040]).astype(mmnp)
    if fp8:
        e4m3 = ml_dtypes.float8_e4m3
        R1s = R1f[0:1024] * np.float32(R1_SCALE)              # [1024, 1024]
        # [qq, s, p, half, n'] -> [p, half, qq, s, n']
        R1q = R1s.reshape(4, 2, P, 2, 512).transpose(2, 3, 0, 1, 4)
        R1hq = R1q.astype(e4m3)
        R1lq = (R1q - R1hq.astype(np.float32)).astype(e4m3)
        R1h = np.ascontiguousarray(R1hq)
        R1l = np.ascontiguousarray(R1lq)
        R1x = np.ascontiguousarray(
            R1f[1024:1040] * np.float32(R1_SCALE)).astype(mmnp)
    else:
        R1 = np.ascontiguousarray(R1f[0:1024]).astype(mmnp)
        R1x = np.ascontiguousarray(R1f[1024:1040]).astype(mmnp)
    BBf = np.concatenate([bw2, bb2[None]], axis=0)            # [65, 1280]
    BBp = np.ascontiguousarray(
        np.concatenate([BBf[:, :1024] * s16, BBf[:, 1024:] * sC], axis=1)
    ).astype(mmnp)

    # replicated-column weights: output partition (c8,w) = 16*c8 + w
    W0s = W0 * np.float32(1.0 / np.sqrt(128.0))               # [128, 16]
    W1s = W1 * np.float32(1.0 / 8.0)                          # [64, 16]
    rep = np.arange(1024)
    gsel = (rep // 128) * 8 + (rep % 128) // 16               # c = 8q + c8
    lw1rep = lw1[:, gsel]                                     # [128, 1024]
    W0rep = np.tile(W0s, (1, 8))                              # [128, 128]
    W1rep = np.tile(W1s, (1, 8))                              # [64, 128]
    # layout must match HCOL & friends in _emit (embT in cols 0:512,
    # filled per core below)
    wpk = np.zeros((128, 1872), f32)
    hcol = [512, 640, 976, 1104, 1232, 1360, 1488, 1616]
    for q in range(8):
        wpk[:, hcol[q]:hcol[q] + 128] = lw1rep[:, 128 * q:128 * (q + 1)]
        wpk[:, 768 + q] = lb1[8 * q + np.arange(128) // 16]
    wpk[0:64, 776] = bb1
    wpk[:, 784:848] = bw1
    wpk[:, 848:976] = W0rep
    wpk[0:64, 1744:1872] = W1rep
    wpk = wpk.astype(mmnp)

    skip_lb2 = not bool(np.any(lb2))

    in_maps = []
    for i in range(N_CORES):
        sl = slice(i * BC, (i + 1) * BC)
        fs = feat[sl]
        featT = np.ascontiguousarray(
            np.concatenate(
                [fs[:, :128], fs[:, 128::3], fs[:, 129::3], fs[:, 130::3]],
                axis=1).T.astype(mmnp))                       # [320, BC]
        wpk_i = wpk.copy()
        wpk_i[:, 0:512] = node_emb[sl].T.astype(mmnp)
        m = {
            "featT": featT,
            "wpk": np.ascontiguousarray(wpk_i), "BBp": BBp,
            "R0": R0,
        }
        if fp8:
            m["R1h"] = R1h
            m["R1l"] = R1l
        else:
            m["R1"] = R1
        if not skip_lb2:
            m["R0x"] = R0x
            m["R1x"] = R1x
        in_maps.append(m)
    return in_maps, skip_lb2, fp8


def _unpack_output(buf):
    """[B, 4352] packed columns -> [B, 80, 80] float32."""
    bf = buf.astype(np.float32)
    n = bf.shape[0]
    out3 = np.zeros((n, 80, 80), np.float32)
    out3[:, 0:16, 0:32] = bf[:, 0:512].reshape(n, 16, 32)
    out3[:, 16:32, 0:32] = bf[:, 512:1024].reshape(n, 16, 32)
    p11 = bf[:, 1024:1280].reshape(n, 16, 16)
    for i in range(3):
        out3[:, 32 + i::3, 32 + i::3] = p11
    for k in range(3):
        out3[:, 0:32, 32 + k::3] = \
            bf[:, 1280 + 512 * k:1792 + 512 * k].reshape(n, 32, 16)
    for i in range(3):
        out3[:, 32 + i::3, 0:32] = \
            bf[:, 2816 + 512 * i:3328 + 512 * i].reshape(n, 16, 32)
    return out3


def run(inputs, mode=None, trace=False):
    """Build (cached), run on 8 cores, gather. Returns (out, results)."""
    mode = mode or MM_MODE
    in_maps, skip_lb2, fp8 = _prepare(inputs, mode)
    key = (mode, skip_lb2, fp8)
    if key not in _CACHE:
        _CACHE[key] = _build_program(mode, skip_lb2, fp8)
    nc = _CACHE[key]

    from concourse.bass_utils import run_bass_kernel_spmd
    for attempt in range(3):
        res = run_bass_kernel_spmd(nc, in_maps, list(range(N_CORES)),
                                   trace=trace)
        buf = np.concatenate([res.results[i]["out"] for i in range(N_CORES)],
                             axis=0)
        out = _unpack_output(buf)
        # guard against a rare transport/device flake observed to return NaN
        # payloads; correct runs are deterministic, so retrying is safe
        if not np.isnan(out).any():
            return out, res
    return out, res


def kernel(**inputs):
    out, _ = run(inputs)
    return out



# revision 44
# speedup vs baseline: 1.3870x; 1.0134x over previous
"""Trainium2 Bass kernel for nn_Expansion (e3nn-style tensor-product expansion).

Math reformulation (verified against the jax reference):
  h   = silu(node_emb @ lw1 + lb1)                         [B,64]
  hb  = silu(node_emb @ bw1 + bb1)                         [B,64]
  x0  = feat[:,:128] @ W0 / sqrt(128)                      [B,16]
  x1k = feat[:,128+k::3] @ W1 / 8          (k=0,1,2)       [B,16]

The per-sample path contractions with wpath = (h @ lw2 + lb2) are a batched
bilinear form

  r[b,p] = sum_{c,w} h'[b,c] x[b,w] M[(c,w), p],   h' = [h, 1]

computed as a plain matmul over the outer product z[b,(c,w)] = h'[b,c]*x[b,w]
(K = 64*16 = 1024 in 8 chunks of 128, + a K=65 chunk for the bias MLP @ BB)
against reshaped weight matrices M built from lw2/bw2 on the host.  This
avoids materializing w = h@lw2 ([B,36864], ~600 MB) entirely.

v2 layout decisions (all driven by the instruction-cost timeline model):
  - The partition-replicated tiles hbc[q][(c8,w),b] = h[8q+c8,b] and
    xbc[t][(c8,w),b] = x_t[w,b] are produced DIRECTLY by the MLP/projection
    matmuls using host-replicated weight columns (lw1rep/W0rep/W1rep), so no
    separate selection-matrix matmuls or extra copies exist.  ACT applies
    silu (with replicated bias) straight from PSUM into bf16 SBUF tiles.
  - All matmuls run in bf16 (fp32 matmuls cost 4x); inputs ship as bf16.
  - The device output is the raw concatenation of the 9 PSUM bank groups per
    sample, [BC, 4352] bf16 — a fixed column permutation of the nonzero
    entries of the [80,80] block matrix.  The host scatters it into the
    final [B,80,80] float32 (incl. the blk11 diagonal triplication and the
    structural zeros), so the device does zero strided/duplicated writes.
  - First b-tile interleaves its three z0-consuming bank groups q-major so
    the PE consumption rate (3 matmuls / z0 chunk) matches ACT's silu
    production rate during warmup.

Sharding: pure data parallel, batch 4096 -> 8 cores x 512.  Weights
replicated; no cross-device communication.

v3: phases 2/3 (blk01/blk10, 69% of the matmul rows) run as fp8e4
DoubleRow matmuls.  Each pair of K-chunks (2q, 2q+1) becomes two
K=256 DoubleRow matmuls -- one against an e4m3 R1_hi pair, one against
the e4m3 residual R1_lo pair -- at 0.5 cycles/row each, halving the
phase-2/3 PE time.  The R side is exact to ~6e-4 (hi+lo split, scaled
x4096 to clear the e4m3 denormal floor; the 2^-12 descale is folded
into the PSUM->SBUF copies).  The z side is a single e4m3 rounding
(sigma ~2.6e-2) applied only to the blk01/blk10 paths (42.6% of output
norm^2), leaving total rel err ~1.8e-2 vs the 2e-2 gate (verified
bit-exact RNE conversion on device).  blk00/blk11 (phase 1) stay bf16.
"""

import sys

import numpy as np

sys.path.insert(0, "/opt/trn_rl_repo")

import ml_dtypes  # noqa: E402

B_TOTAL = 4096
N_CORES = 8
BC = B_TOTAL // N_CORES  # 512 samples per core
P = 128
NB = BC // P  # 4 b-tiles per core
C3 = 1.0 / np.sqrt(3.0)
NCOL = 4352  # packed device output columns per sample

# matmul dtype mode: "bf16" | "f32"
MM_MODE = "bf16"
# fp8e4 DoubleRow for phases 2/3 (blk01/blk10); only in bf16 mode
FP8_P23 = True
R1_SCALE = 4096.0  # lifts e4m3(R1) out of the denormal floor
R1_DESCALE = 1.0 / R1_SCALE
N_WARM = 16   # PE warmup dummies before prep
N_FILL = 6    # PE fillers covering the silu->z_mul latency
PSUM_PREP = 3
PSUM_MAIN = 5

_CACHE = {}


def _np_mm_dtype(mode):
    return ml_dtypes.bfloat16 if mode == "bf16" else np.float32


def _build_program(mode, skip_lb2, fp8):
    import concourse.tile as tile
    from concourse import bacc, mybir

    F32 = mybir.dt.float32
    MM = mybir.dt.bfloat16 if mode == "bf16" else mybir.dt.float32
    AF = mybir.ActivationFunctionType

    nc = bacc.Bacc("TRN2", target_bir_lowering=False, debug=False,
                   num_devices=N_CORES)

    t = {}
    t["featT"] = nc.dram_tensor("featT", [320, BC], MM, kind="ExternalInput").ap()
    t["wpk"] = nc.dram_tensor("wpk", [P, 1872], MM, kind="ExternalInput").ap()
    t["BBp"] = nc.dram_tensor("BBp", [65, 1280], MM, kind="ExternalInput").ap()
    t["R0"] = nc.dram_tensor("R0", [1024, 1280], MM, kind="ExternalInput").ap()
    if fp8:
        FP8 = mybir.dt.float8e4
        # [p, col-half, qq, subtile, n'] -- hi and e4m3 residual pairs
        t["R1h"] = nc.dram_tensor("R1h", [P, 2, 4, 2, 512], FP8,
                                  kind="ExternalInput").ap()
        t["R1l"] = nc.dram_tensor("R1l", [P, 2, 4, 2, 512], FP8,
                                  kind="ExternalInput").ap()
    else:
        t["R1"] = nc.dram_tensor("R1", [1024, 1024], MM, kind="ExternalInput").ap()
    if not skip_lb2:
        t["R0x"] = nc.dram_tensor("R0x", [16, 1280], MM, kind="ExternalInput").ap()
        t["R1x"] = nc.dram_tensor("R1x", [16, 1024], MM, kind="ExternalInput").ap()
    t["out"] = nc.dram_tensor("out", [BC, NCOL], MM, kind="ExternalOutput").ap()

    with tile.TileContext(nc) as tc:
        _emit(tc, t, mode, skip_lb2, fp8, mybir, MM, F32, AF)

    nc.compile()
    return nc


def _emit(tc, t, mode, skip_lb2, fp8, mybir, MM, F32, AF):
    nc = tc.nc
    FP8 = mybir.dt.float8e4
    from contextlib import ExitStack

    with ExitStack() as ctx:
        wpool = ctx.enter_context(tc.tile_pool(name="weights", bufs=1))
        apool = ctx.enter_context(tc.tile_pool(name="acts", bufs=1))
        zpool = ctx.enter_context(tc.tile_pool(name="z", bufs=1))
        opool = ctx.enter_context(tc.tile_pool(name="outs", bufs=3))
        prep_psum = ctx.enter_context(tc.tile_pool(name="prep_psum", bufs=PSUM_PREP, space="PSUM"))
        main_psum = ctx.enter_context(tc.tile_pool(name="main_psum", bufs=PSUM_MAIN, space="PSUM"))

        # ---- SBUF tiles ----
        # wpk column layout: [embT(512) | lw1rep q0 | q1 | lb1rep(8) |
        #   bb1(1) | pad(7) | bw1(64) | W0rep(128) || lw1rep q2..q7 |
        #   W1rep(128, rows 0:64)]
        # The prefix [0:976] is everything the first prep matmuls need.
        wpk_sb = wpool.tile([P, 1872], MM, tag="wpk")
        BB_sb = wpool.tile([65, 1280], MM, tag="BBp")
        R0_sb = wpool.tile([P, 8, 1280], MM, tag="R0")
        if fp8:
            R1h_sb = wpool.tile([P, 2, 4, 2, 512], FP8, tag="R1h")
            R1l_sb = wpool.tile([P, 2, 4, 2, 512], FP8, tag="R1l")
        else:
            R1_sb = wpool.tile([P, 8, 1024], MM, tag="R1")
        if not skip_lb2:
            R0x_sb = wpool.tile([16, 1280], MM, tag="R0x")
            R1x_sb = wpool.tile([16, 1024], MM, tag="R1x")

        feats_sb = apool.tile([P, BC], MM, tag="feats")
        featv_sb = apool.tile([64, 3, BC], MM, tag="featv")
        hbp_sb = apool.tile([65, BC], MM, tag="hbp")
        hbc = [apool.tile([P, BC], MM, name=f"hbc{q}", tag=f"hbc{q}")
               for q in range(8)]
        xbc = [apool.tile([P, BC], MM, name=f"xbc{t_}", tag=f"xbc{t_}")
               for t_ in range(4)]

        # ---- input DMAs, ordered by first consumer ----
        # R0 arrives q-chunk-major to match the phase-1 q-major consumption;
        # R1 follows in halves (phase 2/3 consume it much later)
        r0v = t["R0"].rearrange("(q p) n -> p q n", p=P)
        nc.sync.dma_start(wpk_sb[:, 0:784], t["wpk"][:, 0:784])
        nc.sync.dma_start(feats_sb[:], t["featT"][0:128])
        nc.sync.dma_start(wpk_sb[:, 784:976], t["wpk"][:, 784:976])
        nc.sync.dma_start(wpk_sb[:, 976:1872], t["wpk"][:, 976:1872])
        nc.sync.dma_start(R0_sb[:, 0, :], r0v[:, 0, :])
        nc.sync.dma_start(R0_sb[:, 1, :], r0v[:, 1, :])
        nc.sync.dma_start(R0_sb[:, 2, :], r0v[:, 2, :])
        nc.sync.dma_start(R0_sb[:, 3, :], r0v[:, 3, :])
        nc.sync.dma_start(BB_sb[:], t["BBp"][:])
        nc.sync.dma_start(R0_sb[:, 4, :], r0v[:, 4, :])
        nc.sync.dma_start(featv_sb[:],
                          t["featT"][128:320].rearrange("(k p) b -> p k b", k=3))
        for q in range(5, 8):
            nc.sync.dma_start(R0_sb[:, q, :], r0v[:, q, :])
        if fp8:
            nc.sync.dma_start(R1h_sb[:, 0], t["R1h"][:, 0])
            nc.sync.dma_start(R1l_sb[:, 0], t["R1l"][:, 0])
            nc.sync.dma_start(R1h_sb[:, 1], t["R1h"][:, 1])
            nc.sync.dma_start(R1l_sb[:, 1], t["R1l"][:, 1])
        else:
            r1v = t["R1"].rearrange("(q p) n -> p q n", p=P)
            nc.sync.dma_start(R1_sb[:, 0:4, 0:512], r1v[:, 0:4, 0:512])
            nc.sync.dma_start(R1_sb[:, 4:8, 0:512], r1v[:, 4:8, 0:512])
            nc.sync.dma_start(R1_sb[:, 0:4, 512:1024], r1v[:, 0:4, 512:1024])
            nc.sync.dma_start(R1_sb[:, 4:8, 512:1024], r1v[:, 4:8, 512:1024])
        if not skip_lb2:
            nc.sync.dma_start(R0x_sb[:], t["R0x"][:])
            nc.sync.dma_start(R1x_sb[:], t["R1x"][:])

        # ---- PE warmup: dummy matmuls on a zeroed tile keep the PE busy
        # (and its p-state ramping) while the first input DMAs land; results
        # go to a scratch PSUM bank that is overwritten before any real use.
        warm_sb = apool.tile([P, P], MM, tag="warm")
        nc.vector.memset(warm_sb[:], 0.0)
        # preload the ACT activation table so the first real silu doesn't pay
        # the ~1.3us table-load latency (separate tile: no dep on warm_sb)
        tbl_sb = apool.tile([1, 4], MM, tag="tbl")
        nc.vector.memset(tbl_sb[:], 0.0)
        nc.scalar.activation(tbl_sb[0:1, 0:1], tbl_sb[0:1, 2:3], AF.Silu)
        pwarm = prep_psum.tile([P, P], F32, tag="pp")
        for _ in range(N_WARM):
            nc.tensor.matmul(pwarm[:], lhsT=warm_sb[:], rhs=warm_sb[:],
                             start=True, stop=True)

        # ---- prep emitters ----
        z = [[None] * 8 for _ in range(4)]
        zp = [[None] * 4 for _ in range(4)]  # fp8 chunk-pair tiles, tdx>=1
        HCOL = [512, 640, 976, 1104, 1232, 1360, 1488, 1616]

        def h_mm(q):
            # hbc[q][(c8,w),b] = silu((lw1rep_q)^T emb + lb1rep_q) = h[8q+c8,b]
            phq = prep_psum.tile([P, BC], F32, name=f"ph{q}", tag="pp")
            nc.tensor.matmul(phq[:], lhsT=wpk_sb[:, HCOL[q]:HCOL[q] + P],
                             rhs=wpk_sb[:, 0:512], start=True, stop=True)
            nc.scalar.activation(hbc[q][:], phq[:], AF.Silu,
                                 bias=wpk_sb[:, 768 + q:769 + q])

        def x_mm(tdx):
            # xbc[t][(c8,w),b] = x_t[w,b] via column-replicated W0/W1
            # (copy on DVE — ACT is saturated by the silu chain during prep)
            pxt = prep_psum.tile([P, BC], F32, name=f"px{tdx}", tag="pp")
            if tdx == 0:
                nc.tensor.matmul(pxt[:], lhsT=wpk_sb[:, 848:976],
                                 rhs=feats_sb[:], start=True, stop=True)
            else:
                nc.tensor.matmul(pxt[:], lhsT=wpk_sb[0:64, 1744:1872],
                                 rhs=featv_sb[:, tdx - 1, :],
                                 start=True, stop=True)
            # ACT copy: DVE is the zp-mul critical path into phase 2
            nc.scalar.copy(xbc[tdx][:], pxt[:])

        def z_mul(tdx, q):
            if fp8 and tdx >= 1:
                # e4m3 z, written into subtile slot q%2 of the (2q', 2q'+1)
                # DoubleRow pair tile (single RNE rounding off the bf16 mul)
                qq = q // 2
                if zp[tdx][qq] is None:
                    zp[tdx][qq] = zpool.tile([P, 2, BC], FP8,
                                             name=f"zp{tdx}_{qq}",
                                             tag=f"zp{tdx}_{qq}")
                nc.vector.tensor_mul(out=zp[tdx][qq][:, q % 2, :],
                                     in0=hbc[q][:], in1=xbc[tdx][:])
                return
            zt = zpool.tile([P, BC], MM, name=f"z{tdx}_{q}", tag=f"z{tdx}_{q}")
            nc.vector.tensor_mul(out=zt[:], in0=hbc[q][:], in1=xbc[tdx][:])
            z[tdx][q] = zt

        h_mm(0)
        x_mm(0)
        h_mm(1)

        # hbp[c,b] = silu(bw1^T emb + bb1), plus a ones row for the bb2 path
        ph = prep_psum.tile([64, BC], F32, tag="pp")
        nc.tensor.matmul(ph[:], lhsT=wpk_sb[:, 784:848], rhs=wpk_sb[:, 0:512],
                         start=True, stop=True)
        nc.scalar.activation(hbp_sb[0:64, :], ph[:], AF.Silu,
                             bias=wpk_sb[0:64, 776:777])
        nc.gpsimd.memset(hbp_sb[64:65, :], 1.0)

        # fillers: cover the silu->z_mul latency before the first z matmul
        for _ in range(N_FILL):
            nc.tensor.matmul(pwarm[:], lhsT=warm_sb[:], rhs=warm_sb[:],
                             start=True, stop=True)

        # ---- main accumulation groups, group-type-major ----
        # phase 1 (needs only R0): per b-tile g0..g2 = z0 @ R0 cols
        # (0:512 | 512:1024 | 1024:1280) + the BB bias chunk, q-major
        # interleaved across the three banks.  The j==0 pass interleaves the
        # remaining prep matmuls so the PE tracks ACT's silu cadence.
        # phase 2 (R1[:, 0:512]): g3..g5 = z[1+k] @ R1 left  (blk01)
        # phase 3 (R1[:, 512:1024]): g6..g8 = z[1+i] @ R1 right (blk10)
        def copy_out(eng, dst_ap, src_ap):
            # blk01/blk10 columns keep the 2^12 R1 scale in bf16 (relative
            # precision is scale-free); the host descales during unpack
            if eng == "a":
                nc.scalar.copy(dst_ap, src_ap)
            elif eng == "p":
                nc.gpsimd.tensor_copy(out=dst_ap, in_=src_ap)
            else:
                nc.vector.tensor_copy(out=dst_ap, in_=src_ap)

        out_t = [opool.tile([P, NCOL], MM, name=f"out_t{j}", tag=f"out_t{j}")
                 for j in range(NB)]
        P1_ENG = ['aaa', 'aaa', 'aaa', 'add']  # per-j engines for g0/g1/g2
        P23_ENG = ['adadad', 'dadada', 'adadad', 'dadada']  # per-j g3..g8

        def phase1(j, final):
            bsl = slice(P * j, P * (j + 1))
            gcols = [(0, 512), (512, 1024), (1024, 1280)]
            if final:
                # sequential groups with per-group writeback: only the last
                # (smallest) group's copy+DMA trail the final matmul
                for g, (c0, c1) in enumerate(gcols):
                    pg = main_psum.tile([P, 512], F32, name=f"pg{j}_{g}",
                                        tag="mp")
                    psl = pg[:, 0:c1 - c0]
                    for q in range(8):
                        nc.tensor.matmul(psl, lhsT=z[0][q][:, bsl],
                                         rhs=R0_sb[:, q, c0:c1],
                                         start=(q == 0), stop=False)
                    if not skip_lb2:
                        nc.tensor.matmul(psl, lhsT=xbc[0][0:16, bsl],
                                         rhs=R0x_sb[:, c0:c1],
                                         start=False, stop=False)
                    nc.tensor.matmul(psl, lhsT=hbp_sb[:, bsl],
                                     rhs=BB_sb[:, c0:c1],
                                     start=False, stop=True)
                    # alternate copy engines and DMA queues so the
                    # trailing writebacks generate descriptors in parallel
                    copy_out("ada"[g], out_t[j][:, c0:c1], psl)
                    dma_eng = [nc.scalar, nc.gpsimd, nc.sync][g]
                    dma_eng.dma_start(t["out"][bsl, c0:c1],
                                      out_t[j][:, c0:c1])
                return
            pg = [main_psum.tile([P, 512], F32, name=f"pg{j}_{g}", tag="mp")
                  for g in range(3)]
            gsl = [pg[0][:], pg[1][:], pg[2][:, 0:256]]
            for q in range(8):
                if j == 0:
                    z_mul(0, q)
                for g in range(3):
                    c0, c1 = gcols[g]
                    nc.tensor.matmul(gsl[g], lhsT=z[0][q][:, bsl],
                                     rhs=R0_sb[:, q, c0:c1],
                                     start=(q == 0), stop=False)
                if j == 0:
                    # the next replication matmul comes AFTER this q's main
                    # matmuls so a late weight DMA can't block them in-order
                    if q < 6:
                        h_mm(q + 2)
                    else:
                        x_mm(1 if q == 6 else 2)
            if j == 0:
                x_mm(3)
                for tdx in range(1, 4):
                    for q in range(8):
                        z_mul(tdx, q)
            if not skip_lb2:
                for g in range(3):
                    c0, c1 = gcols[g]
                    nc.tensor.matmul(gsl[g], lhsT=xbc[0][0:16, bsl],
                                     rhs=R0x_sb[:, c0:c1],
                                     start=False, stop=False)
            for g in range(3):
                c0, c1 = gcols[g]
                nc.tensor.matmul(gsl[g], lhsT=hbp_sb[:, bsl],
                                 rhs=BB_sb[:, c0:c1], start=False, stop=True)
            for g in range(3):
                c0, c1 = gcols[g]
                copy_out(P1_ENG[j][g], out_t[j][:, c0:c1], gsl[g])
            nc.sync.dma_start(t["out"][bsl, 0:1280], out_t[j][:, 0:1280])

        for j in range(NB - 1):
            phase1(j, final=False)

        DRM = mybir.MatmulPerfMode.DoubleRow

        def p23_group(phase, j, mi):
            bsl = slice(P * j, P * (j + 1))
            rc = (0, 512) if phase == 0 else (512, 1024)
            m = 3 * phase + mi
            tdx = 1 + mi
            pgm = main_psum.tile([P, 512], F32, name=f"pm{j}_{m}", tag="mp")
            if fp8:
                for qq in range(4):
                    nc.tensor.matmul(pgm[:], lhsT=zp[tdx][qq][:, :, bsl],
                                     rhs=R1h_sb[:, phase, qq, :, :],
                                     start=(qq == 0), stop=False,
                                     perf_mode=DRM)
                    nc.tensor.matmul(pgm[:], lhsT=zp[tdx][qq][:, :, bsl],
                                     rhs=R1l_sb[:, phase, qq, :, :],
                                     start=False, stop=(skip_lb2 and qq == 3),
                                     perf_mode=DRM)
            else:
                for q in range(8):
                    last = skip_lb2 and q == 7
                    nc.tensor.matmul(pgm[:], lhsT=z[tdx][q][:, bsl],
                                     rhs=R1_sb[:, q, rc[0]:rc[1]],
                                     start=(q == 0), stop=last)
            if not skip_lb2:
                nc.tensor.matmul(pgm[:], lhsT=xbc[tdx][0:16, bsl],
                                 rhs=R1x_sb[:, rc[0]:rc[1]],
                                 start=False, stop=True)
            c0 = 1280 + 512 * m
            copy_out(P23_ENG[j][m], out_t[j][:, c0:c0 + 512], pgm[:])

        # phase 2 runs mi-major so its first groups only need zp[1][*]
        # (the zp muls for tdx 2/3 are still draining on DVE then);
        # phase 3 stays j-major
        for j, mi in [(j, mi) for mi in range(3) for j in range(NB)]:
            p23_group(0, j, mi)
            if mi == 2:
                bsl = slice(P * j, P * (j + 1))
                nc.sync.dma_start(t["out"][bsl, 1280:2816],
                                  out_t[j][:, 1280:2816])
        for j in range(NB):
            for mi in range(3):
                p23_group(1, j, mi)
            bsl = slice(P * j, P * (j + 1))
            nc.sync.dma_start(t["out"][bsl, 2816:4352],
                              out_t[j][:, 2816:4352])

        phase1(NB - 1, final=True)


def _prepare(inputs, mode):
    f32 = np.float32
    feat = np.ascontiguousarray(np.asarray(inputs["feat"], dtype=f32))
    node_emb = np.ascontiguousarray(np.asarray(inputs["node_emb"], dtype=f32))
    W0 = np.asarray(inputs["W0"], f32)
    W1 = np.asarray(inputs["W1"], f32)
    lw1 = np.asarray(inputs["lw1"], f32)
    lb1 = np.asarray(inputs["lb1"], f32)
    lw2 = np.asarray(inputs["lw2"], f32)
    lb2 = np.asarray(inputs["lb2"], f32)
    bw1 = np.asarray(inputs["bw1"], f32)
    bb1 = np.asarray(inputs["bb1"], f32)
    bw2 = np.asarray(inputs["bw2"], f32)
    bb2 = np.asarray(inputs["bb2"], f32)

    mmnp = _np_mm_dtype(mode)
    s16 = np.float32(1.0 / 16.0)
    sC = np.float32(C3 / 16.0)

    # weight matrices for the main contraction, path scales folded in
    lw2p = np.concatenate([lw2, lb2[None]], axis=0)           # [65, 36864]
    M00 = lw2p[:, :16384].reshape(1040, 1024) * s16
    M11 = lw2p[:, 16384:20480].reshape(1040, 256) * sC
    M01 = lw2p[:, 20480:28672].reshape(1040, 512) * sC
    M10 = lw2p[:, 28672:36864].reshape(1040, 512) * sC
    R0f = np.concatenate([M00, M11], axis=1)                  # [1040, 1280]
    R1f = np.concatenate([M01, M10], axis=1)                  # [1040, 1024]
    fp8 = FP8_P23 and mode == "bf16"
    R0 = np.ascontiguousarray(R0f[0:1024]).astype(mmnp)
    R0x = np.ascontiguousarray(R0f[1024:1040]).astype(mmnp)
    if fp8:
        e4m3 = ml_dtypes.float8_e4m3
        R1s = R1f[0:1024] * np.float32(R1_SCALE)              # [1024, 1024]
        # [qq, s, p, half, n'] -> [p, half, qq, s, n']
        R1q = R1s.reshape(4, 2, P, 2, 512).transpose(2, 3, 0, 1, 4)
        R1hq = R1q.astype(e4m3)
        R1lq = (R1q - R1hq.astype(np.float32)).astype(e4m3)
        R1h = np.ascontiguousarray(R1hq)
        R1l = np.ascontiguousarray(R1lq)
        R1x = np.ascontiguousarray(
            R1f[1024:1040] * np.float32(R1_SCALE)).astype(mmnp)
    else:
        R1 = np.ascontiguousarray(R1f[0:1024]).astype(mmnp)
        R1x = np.ascontiguousarray(R1f[1024:1040]).astype(mmnp)
    BBf = np.concatenate([bw2, bb2[None]], axis=0)            # [65, 1280]
    BBp = np.ascontiguousarray(
        np.concatenate([BBf[:, :1024] * s16, BBf[:, 1024:] * sC], axis=1)
    ).astype(mmnp)

    # replicated-column weights: output partition (c8,w) = 16*c8 + w
    W0s = W0 * np.float32(1.0 / np.sqrt(128.0))               # [128, 16]
    W1s = W1 * np.float32(1.0 / 8.0)                          # [64, 16]
    rep = np.arange(1024)
    gsel = (rep // 128) * 8 + (rep % 128) // 16               # c = 8q + c8
    lw1rep = lw1[:, gsel]                                     # [128, 1024]
    W0rep = np.tile(W0s, (1, 8))                              # [128, 128]
    W1rep = np.tile(W1s, (1, 8))                              # [64, 128]
    # layout must match HCOL & friends in _emit (embT in cols 0:512,
    # filled per core below)
    wpk = np.zeros((128, 1872), f32)
    hcol = [512, 640, 976, 1104, 1232, 1360, 1488, 1616]
    for q in range(8):
        wpk[:, hcol[q]:hcol[q] + 128] = lw1rep[:, 128 * q:128 * (q + 1)]
        wpk[:, 768 + q] = lb1[8 * q + np.arange(128) // 16]
    wpk[0:64, 776] = bb1
    wpk[:, 784:848] = bw1
    wpk[:, 848:976] = W0rep
    wpk[0:64, 1744:1872] = W1rep
    wpk = wpk.astype(mmnp)

    skip_lb2 = not bool(np.any(lb2))

    in_maps = []
    for i in range(N_CORES):
        sl = slice(i * BC, (i + 1) * BC)
        fs = feat[sl]
        featT = np.ascontiguousarray(
            np.concatenate(
                [fs[:, :128], fs[:, 128::3], fs[:, 129::3], fs[:, 130::3]],
                axis=1).T.astype(mmnp))                       # [320, BC]
        wpk_i = wpk.copy()
        wpk_i[:, 0:512] = node_emb[sl].T.astype(mmnp)
        m = {
            "featT": featT,
            "wpk": np.ascontiguousarray(wpk_i), "BBp": BBp,
            "R0": R0,
        }
        if fp8:
            m["R1h"] = R1h
            m["R1l"] = R1l
        else:
            m["R1"] = R1
        if not skip_lb2:
            m["R0x"] = R0x
            m["R1x"] = R1x
        in_maps.append(m)
    return in_maps, skip_lb2, fp8


def _unpack_output(buf, fp8):
    """[B, 4352] packed columns -> [B, 80, 80] float32.

    In fp8 mode, columns 1280: (blk01/blk10) carry the 2^12 R1 scale;
    descaled here.
    """
    bf = buf.astype(np.float32)
    n = bf.shape[0]
    if fp8:
        bf[:, 1280:] *= np.float32(R1_DESCALE)
    out3 = np.zeros((n, 80, 80), np.float32)
    out3[:, 0:16, 0:32] = bf[:, 0:512].reshape(n, 16, 32)
    out3[:, 16:32, 0:32] = bf[:, 512:1024].reshape(n, 16, 32)
    p11 = bf[:, 1024:1280].reshape(n, 16, 16)
    for i in range(3):
        out3[:, 32 + i::3, 32 + i::3] = p11
    for k in range(3):
        out3[:, 0:32, 32 + k::3] = \
            bf[:, 1280 + 512 * k:1792 + 512 * k].reshape(n, 32, 16)
    for i in range(3):
        out3[:, 32 + i::3, 0:32] = \
            bf[:, 2816 + 512 * i:3328 + 512 * i].reshape(n, 16, 32)
    return out3


def run(inputs, mode=None, trace=False):
    """Build (cached), run on 8 cores, gather. Returns (out, results)."""
    mode = mode or MM_MODE
    in_maps, skip_lb2, fp8 = _prepare(inputs, mode)
    key = (mode, skip_lb2, fp8)
    if key not in _CACHE:
        _CACHE[key] = _build_program(mode, skip_lb2, fp8)
    nc = _CACHE[key]

    from concourse.bass_utils import run_bass_kernel_spmd
    for attempt in range(3):
        res = run_bass_kernel_spmd(nc, in_maps, list(range(N_CORES)),
                                   trace=trace)
        buf = np.concatenate([res.results[i]["out"] for i in range(N_CORES)],
                             axis=0)
        out = _unpack_output(buf, fp8)
        # guard against a rare transport/device flake observed to return NaN
        # payloads; correct runs are deterministic, so retrying is safe
        if not np.isnan(out).any():
            return out, res
    return out, res


def kernel(**inputs):
    out, _ = run(inputs)
    return out



# revision 53
# speedup vs baseline: 1.4063x; 1.0139x over previous
"""Trainium2 Bass kernel for nn_Expansion (e3nn-style tensor-product expansion).

Math reformulation (verified against the jax reference):
  h   = silu(node_emb @ lw1 + lb1)                         [B,64]
  hb  = silu(node_emb @ bw1 + bb1)                         [B,64]
  x0  = feat[:,:128] @ W0 / sqrt(128)                      [B,16]
  x1k = feat[:,128+k::3] @ W1 / 8          (k=0,1,2)       [B,16]

The per-sample path contractions with wpath = (h @ lw2 + lb2) are a batched
bilinear form

  r[b,p] = sum_{c,w} h'[b,c] x[b,w] M[(c,w), p],   h' = [h, 1]

computed as a plain matmul over the outer product z[b,(c,w)] = h'[b,c]*x[b,w]
(K = 64*16 = 1024 in 8 chunks of 128, + a K=65 chunk for the bias MLP @ BB)
against reshaped weight matrices M built from lw2/bw2 on the host.  This
avoids materializing w = h@lw2 ([B,36864], ~600 MB) entirely.

v2 layout decisions (all driven by the instruction-cost timeline model):
  - The partition-replicated tiles hbc[q][(c8,w),b] = h[8q+c8,b] and
    xbc[t][(c8,w),b] = x_t[w,b] are produced DIRECTLY by the MLP/projection
    matmuls using host-replicated weight columns (lw1rep/W0rep/W1rep), so no
    separate selection-matrix matmuls or extra copies exist.  ACT applies
    silu (with replicated bias) straight from PSUM into bf16 SBUF tiles.
  - All matmuls run in bf16 (fp32 matmuls cost 4x); inputs ship as bf16.
  - The device output is the raw concatenation of the 9 PSUM bank groups per
    sample, [BC, 4352] bf16 — a fixed column permutation of the nonzero
    entries of the [80,80] block matrix.  The host scatters it into the
    final [B,80,80] float32 (incl. the blk11 diagonal triplication and the
    structural zeros), so the device does zero strided/duplicated writes.
  - First b-tile interleaves its three z0-consuming bank groups q-major so
    the PE consumption rate (3 matmuls / z0 chunk) matches ACT's silu
    production rate during warmup.

Sharding: pure data parallel, batch 4096 -> 8 cores x 512.  Weights
replicated; no cross-device communication.

v3: phases 2/3 (blk01/blk10, 69% of the matmul rows) run as fp8e4
DoubleRow matmuls.  Each pair of K-chunks (2q, 2q+1) becomes two
K=256 DoubleRow matmuls -- one against an e4m3 R1_hi pair, one against
the e4m3 residual R1_lo pair -- at 0.5 cycles/row each, halving the
phase-2/3 PE time.  The R side is exact to ~6e-4 (hi+lo split, scaled
x4096 to clear the e4m3 denormal floor; the 2^-12 descale is folded
into the PSUM->SBUF copies).  The z side is a single e4m3 rounding
(sigma ~2.6e-2) applied only to the blk01/blk10 paths (42.6% of output
norm^2), leaving total rel err ~1.8e-2 vs the 2e-2 gate (verified
bit-exact RNE conversion on device).  blk00/blk11 (phase 1) stay bf16.
"""

import sys

import numpy as np

sys.path.insert(0, "/opt/trn_rl_repo")

import ml_dtypes  # noqa: E402

B_TOTAL = 4096
N_CORES = 8
BC = B_TOTAL // N_CORES  # 512 samples per core
P = 128
NB = BC // P  # 4 b-tiles per core
C3 = 1.0 / np.sqrt(3.0)
NCOL = 4352  # packed device output columns per sample

# matmul dtype mode: "bf16" | "f32"
MM_MODE = "bf16"
# fp8e4 DoubleRow for phases 2/3 (blk01/blk10); only in bf16 mode
FP8_P23 = True
R1_SCALE = 4096.0  # lifts e4m3(R1) out of the denormal floor
R1_DESCALE = 1.0 / R1_SCALE
N_WARM = 16   # PE warmup dummies before prep
N_FILL = 6    # PE fillers covering the silu->z_mul latency
PSUM_PREP = 3
PSUM_MAIN = 5

_CACHE = {}


def _np_mm_dtype(mode):
    return ml_dtypes.bfloat16 if mode == "bf16" else np.float32


def _build_program(mode, skip_lb2, fp8):
    import concourse.tile as tile
    from concourse import bacc, mybir

    F32 = mybir.dt.float32
    MM = mybir.dt.bfloat16 if mode == "bf16" else mybir.dt.float32
    AF = mybir.ActivationFunctionType

    nc = bacc.Bacc("TRN2", target_bir_lowering=False, debug=False,
                   num_devices=N_CORES)

    t = {}
    t["featT"] = nc.dram_tensor("featT", [320, BC], MM, kind="ExternalInput").ap()
    t["wpk"] = nc.dram_tensor("wpk", [P, 1872], MM, kind="ExternalInput").ap()
    t["R0"] = nc.dram_tensor("R0", [1024, 1280], MM, kind="ExternalInput").ap()
    if fp8:
        FP8 = mybir.dt.float8e4
        # bias MLP weights as an (hi, e4m3-residual) subtile pair
        t["BBhl"] = nc.dram_tensor("BBhl", [65, 2, 1280], FP8,
                                   kind="ExternalInput").ap()
        # [p, col-half, qq, subtile, n'] -- hi and e4m3 residual pairs
        t["R1h"] = nc.dram_tensor("R1h", [P, 2, 4, 2, 512], FP8,
                                  kind="ExternalInput").ap()
        t["R1l"] = nc.dram_tensor("R1l", [P, 2, 4, 2, 512], FP8,
                                  kind="ExternalInput").ap()
    else:
        t["BBp"] = nc.dram_tensor("BBp", [65, 1280], MM, kind="ExternalInput").ap()
        t["R1"] = nc.dram_tensor("R1", [1024, 1024], MM, kind="ExternalInput").ap()
    if not skip_lb2:
        t["R0x"] = nc.dram_tensor("R0x", [16, 1280], MM, kind="ExternalInput").ap()
        t["R1x"] = nc.dram_tensor("R1x", [16, 1024], MM, kind="ExternalInput").ap()
    t["out"] = nc.dram_tensor("out", [BC, NCOL], MM, kind="ExternalOutput").ap()

    with tile.TileContext(nc) as tc:
        _emit(tc, t, mode, skip_lb2, fp8, mybir, MM, F32, AF)

    nc.compile()
    return nc


def _emit(tc, t, mode, skip_lb2, fp8, mybir, MM, F32, AF):
    nc = tc.nc
    FP8 = mybir.dt.float8e4
    DRM = mybir.MatmulPerfMode.DoubleRow
    from contextlib import ExitStack

    with ExitStack() as ctx:
        wpool = ctx.enter_context(tc.tile_pool(name="weights", bufs=1))
        apool = ctx.enter_context(tc.tile_pool(name="acts", bufs=1))
        zpool = ctx.enter_context(tc.tile_pool(name="z", bufs=1))
        opool = ctx.enter_context(tc.tile_pool(name="outs", bufs=3))
        prep_psum = ctx.enter_context(tc.tile_pool(name="prep_psum", bufs=PSUM_PREP, space="PSUM"))
        main_psum = ctx.enter_context(tc.tile_pool(name="main_psum", bufs=PSUM_MAIN, space="PSUM"))

        # ---- SBUF tiles ----
        # wpk column layout: [embT(512) | lw1rep q0 | q1 | lb1rep(8) |
        #   bb1(1) | pad(7) | bw1(64) | W0rep(128) || lw1rep q2..q7 |
        #   W1rep(128, rows 0:64)]
        # The prefix [0:976] is everything the first prep matmuls need.
        wpk_sb = wpool.tile([P, 1872], MM, tag="wpk")
        R0_sb = wpool.tile([P, 8, 1280], MM, tag="R0")
        if fp8:
            BB_sb = wpool.tile([65, 2, 1280], FP8, tag="BBhl")
            R1h_sb = wpool.tile([P, 2, 4, 2, 512], FP8, tag="R1h")
            R1l_sb = wpool.tile([P, 2, 4, 2, 512], FP8, tag="R1l")
        else:
            BB_sb = wpool.tile([65, 1280], MM, tag="BBp")
            R1_sb = wpool.tile([P, 8, 1024], MM, tag="R1")
        if not skip_lb2:
            R0x_sb = wpool.tile([16, 1280], MM, tag="R0x")
            R1x_sb = wpool.tile([16, 1024], MM, tag="R1x")

        feats_sb = apool.tile([P, BC], MM, tag="feats")
        featv_sb = apool.tile([64, 3, BC], MM, tag="featv")
        if fp8:
            hbp_sb = apool.tile([65, 2, BC], FP8, name="hbp_sb", tag="hbp")
        else:
            hbp_sb = apool.tile([65, BC], MM, name="hbp_sb", tag="hbp")
        hbc = [apool.tile([P, BC], MM, name=f"hbc{q}", tag=f"hbc{q}")
               for q in range(8)]
        xbc = [apool.tile([P, BC], MM, name=f"xbc{t_}", tag=f"xbc{t_}")
               for t_ in range(4)]

        # ---- input DMAs, ordered by first consumer ----
        # R0 arrives q-chunk-major to match the phase-1 q-major consumption;
        # R1 follows in halves (phase 2/3 consume it much later)
        r0v = t["R0"].rearrange("(q p) n -> p q n", p=P)
        nc.sync.dma_start(wpk_sb[:, 0:784], t["wpk"][:, 0:784])
        nc.sync.dma_start(feats_sb[:], t["featT"][0:128])
        nc.sync.dma_start(wpk_sb[:, 784:976], t["wpk"][:, 784:976])
        nc.sync.dma_start(wpk_sb[:, 976:1872], t["wpk"][:, 976:1872])
        nc.sync.dma_start(R0_sb[:, 0, :], r0v[:, 0, :])
        nc.sync.dma_start(R0_sb[:, 1, :], r0v[:, 1, :])
        nc.sync.dma_start(R0_sb[:, 2, :], r0v[:, 2, :])
        nc.sync.dma_start(R0_sb[:, 3, :], r0v[:, 3, :])
        nc.sync.dma_start(BB_sb[:], t["BBhl"][:] if fp8 else t["BBp"][:])
        nc.sync.dma_start(R0_sb[:, 4, :], r0v[:, 4, :])
        nc.sync.dma_start(featv_sb[:],
                          t["featT"][128:320].rearrange("(k p) b -> p k b", k=3))
        for q in range(5, 8):
            nc.sync.dma_start(R0_sb[:, q, :], r0v[:, q, :])
        if fp8:
            nc.sync.dma_start(R1h_sb[:, 0], t["R1h"][:, 0])
            nc.sync.dma_start(R1l_sb[:, 0], t["R1l"][:, 0])
            nc.sync.dma_start(R1h_sb[:, 1], t["R1h"][:, 1])
            nc.sync.dma_start(R1l_sb[:, 1], t["R1l"][:, 1])
        else:
            r1v = t["R1"].rearrange("(q p) n -> p q n", p=P)
            nc.sync.dma_start(R1_sb[:, 0:4, 0:512], r1v[:, 0:4, 0:512])
            nc.sync.dma_start(R1_sb[:, 4:8, 0:512], r1v[:, 4:8, 0:512])
            nc.sync.dma_start(R1_sb[:, 0:4, 512:1024], r1v[:, 0:4, 512:1024])
            nc.sync.dma_start(R1_sb[:, 4:8, 512:1024], r1v[:, 4:8, 512:1024])
        if not skip_lb2:
            nc.sync.dma_start(R0x_sb[:], t["R0x"][:])
            nc.sync.dma_start(R1x_sb[:], t["R1x"][:])

        # ---- PE warmup: dummy matmuls on a zeroed tile keep the PE busy
        # (and its p-state ramping) while the first input DMAs land; results
        # go to a scratch PSUM bank that is overwritten before any real use.
        warm_sb = apool.tile([P, P], MM, tag="warm")
        nc.vector.memset(warm_sb[:], 0.0)
        # preload the ACT activation table so the first real silu doesn't pay
        # the ~1.3us table-load latency (separate tile: no dep on warm_sb)
        tbl_sb = apool.tile([1, 4], MM, tag="tbl")
        nc.vector.memset(tbl_sb[:], 0.0)
        nc.scalar.activation(tbl_sb[0:1, 0:1], tbl_sb[0:1, 2:3], AF.Silu)
        pwarm = prep_psum.tile([P, P], F32, tag="pp")
        for _ in range(N_WARM):
            nc.tensor.matmul(pwarm[:], lhsT=warm_sb[:], rhs=warm_sb[:],
                             start=True, stop=True)

        # ---- prep emitters ----
        z = [[None] * 8 for _ in range(4)]
        zp = [[None] * 4 for _ in range(4)]  # fp8 chunk-pair tiles, tdx>=1
        HCOL = [512, 640, 976, 1104, 1232, 1360, 1488, 1616]

        def h_mm(q):
            # hbc[q][(c8,w),b] = silu((lw1rep_q)^T emb + lb1rep_q) = h[8q+c8,b]
            phq = prep_psum.tile([P, BC], F32, name=f"ph{q}", tag="pp")
            nc.tensor.matmul(phq[:], lhsT=wpk_sb[:, HCOL[q]:HCOL[q] + P],
                             rhs=wpk_sb[:, 0:512], start=True, stop=True)
            nc.scalar.activation(hbc[q][:], phq[:], AF.Silu,
                                 bias=wpk_sb[:, 768 + q:769 + q])

        def x_mm(tdx):
            # xbc[t][(c8,w),b] = x_t[w,b] via column-replicated W0/W1
            # (copy on DVE — ACT is saturated by the silu chain during prep)
            pxt = prep_psum.tile([P, BC], F32, name=f"px{tdx}", tag="pp")
            if tdx == 0:
                nc.tensor.matmul(pxt[:], lhsT=wpk_sb[:, 848:976],
                                 rhs=feats_sb[:], start=True, stop=True)
            else:
                nc.tensor.matmul(pxt[:], lhsT=wpk_sb[0:64, 1744:1872],
                                 rhs=featv_sb[:, tdx - 1, :],
                                 start=True, stop=True)
            # ACT copy: DVE is the zp-mul critical path into phase 2
            nc.scalar.copy(xbc[tdx][:], pxt[:])

        def z_mul(tdx, q):
            if fp8 and tdx >= 1:
                # e4m3 z, written into subtile slot q%2 of the (2q', 2q'+1)
                # DoubleRow pair tile (single RNE rounding off the bf16 mul)
                qq = q // 2
                if zp[tdx][qq] is None:
                    zp[tdx][qq] = zpool.tile([P, 2, BC], FP8,
                                             name=f"zp{tdx}_{qq}",
                                             tag=f"zp{tdx}_{qq}")
                nc.vector.tensor_mul(out=zp[tdx][qq][:, q % 2, :],
                                     in0=hbc[q][:], in1=xbc[tdx][:])
                return
            zt = zpool.tile([P, BC], MM, name=f"z{tdx}_{q}", tag=f"z{tdx}_{q}")
            nc.vector.tensor_mul(out=zt[:], in0=hbc[q][:], in1=xbc[tdx][:])
            z[tdx][q] = zt

        h_mm(0)
        x_mm(0)
        h_mm(1)

        # hbp[c,b] = silu(bw1^T emb + bb1), plus a ones row for the bb2
        # path.  fp8 mode: e4m3 pair tile, the same values in both DoubleRow
        # subtiles (paired against the host-built BB hi/lo rhs).
        ph = prep_psum.tile([64, BC], F32, tag="pp")
        nc.tensor.matmul(ph[:], lhsT=wpk_sb[:, 784:848], rhs=wpk_sb[:, 0:512],
                         start=True, stop=True)
        if fp8:
            nc.scalar.activation(hbp_sb[0:64, 0, :], ph[:], AF.Silu,
                                 bias=wpk_sb[0:64, 776:777])
            nc.vector.tensor_copy(out=hbp_sb[0:64, 1, :],
                                  in_=hbp_sb[0:64, 0, :])
            nc.gpsimd.memset(hbp_sb[64:65, :, :], 1.0)
        else:
            nc.scalar.activation(hbp_sb[0:64, :], ph[:], AF.Silu,
                                 bias=wpk_sb[0:64, 776:777])
            nc.gpsimd.memset(hbp_sb[64:65, :], 1.0)

        # fillers: cover the silu->z_mul latency before the first z matmul
        for _ in range(N_FILL):
            nc.tensor.matmul(pwarm[:], lhsT=warm_sb[:], rhs=warm_sb[:],
                             start=True, stop=True)

        # ---- main accumulation groups, group-type-major ----
        # phase 1 (needs only R0): per b-tile g0..g2 = z0 @ R0 cols
        # (0:512 | 512:1024 | 1024:1280) + the BB bias chunk, q-major
        # interleaved across the three banks.  The j==0 pass interleaves the
        # remaining prep matmuls so the PE tracks ACT's silu cadence.
        # phase 2 (R1[:, 0:512]): g3..g5 = z[1+k] @ R1 left  (blk01)
        # phase 3 (R1[:, 512:1024]): g6..g8 = z[1+i] @ R1 right (blk10)
        def copy_out(eng, dst_ap, src_ap):
            # blk01/blk10 columns keep the 2^12 R1 scale in bf16 (relative
            # precision is scale-free); the host descales during unpack
            if eng == "a":
                nc.scalar.copy(dst_ap, src_ap)
            elif eng == "p":
                nc.gpsimd.tensor_copy(out=dst_ap, in_=src_ap)
            else:
                nc.vector.tensor_copy(out=dst_ap, in_=src_ap)

        out_t = [opool.tile([P, NCOL], MM, name=f"out_t{j}", tag=f"out_t{j}")
                 for j in range(NB)]
        P1_ENG = ['aaa', 'aaa', 'aaa', 'add']  # per-j engines for g0/g1/g2
        P23_ENG = ['adadad', 'dadada', 'adadad', 'dadada']  # per-j g3..g8

        def phase1(j, final):
            bsl = slice(P * j, P * (j + 1))
            gcols = [(0, 512), (512, 1024), (1024, 1280)]
            if final:
                # sequential groups with per-group writeback: only the last
                # (smallest) group's copy+DMA trail the final matmul
                for g, (c0, c1) in enumerate(gcols):
                    pg = main_psum.tile([P, 512], F32, name=f"pg{j}_{g}",
                                        tag="mp")
                    psl = pg[:, 0:c1 - c0]
                    for q in range(8):
                        nc.tensor.matmul(psl, lhsT=z[0][q][:, bsl],
                                         rhs=R0_sb[:, q, c0:c1],
                                         start=(q == 0), stop=False)
                    if not skip_lb2:
                        nc.tensor.matmul(psl, lhsT=xbc[0][0:16, bsl],
                                         rhs=R0x_sb[:, c0:c1],
                                         start=False, stop=False)
                    if fp8:
                        nc.tensor.matmul(psl, lhsT=hbp_sb[:, :, bsl],
                                         rhs=BB_sb[:, :, c0:c1],
                                         start=False, stop=True,
                                         perf_mode=DRM)
                    else:
                        nc.tensor.matmul(psl, lhsT=hbp_sb[:, bsl],
                                         rhs=BB_sb[:, c0:c1],
                                         start=False, stop=True)
                    # alternate copy engines and DMA queues so the
                    # trailing writebacks generate descriptors in parallel
                    copy_out("ada"[g], out_t[j][:, c0:c1], psl)
                    dma_eng = [nc.scalar, nc.gpsimd, nc.sync][g]
                    dma_eng.dma_start(t["out"][bsl, c0:c1],
                                      out_t[j][:, c0:c1])
                return
            raise AssertionError("non-final phase1 goes through phase1_multi")

        GCOLS = [(0, 512), (512, 1024), (1024, 1280)]

        def phase1_multi(jgs, prep=False):
            # q-major across (b-tile, group-subset) pairs: every R0 chunk is
            # consumed by all listed groups right as its DMA lands, so later
            # b-tiles don't sit behind j0's R0-starved matmuls in the
            # in-order PE stream.  Partial group lists keep the in-flight
            # PSUM banks within the pool.  j0's prep interleave rides along.
            gsls = {}
            for j, gl in jgs:
                for g in gl:
                    c0, c1 = GCOLS[g]
                    pg = main_psum.tile([P, 512], F32, name=f"pg{j}_{g}",
                                        tag="mp")
                    gsls[(j, g)] = pg[:, 0:c1 - c0]
            if prep:
                z_mul(0, 0)
            for q in range(8):
                if prep and q < 7:
                    # produce z0[q+1] one slot ahead of its consumers so the
                    # silu->mul latency never stalls the main matmul stream
                    z_mul(0, q + 1)
                for j, gl in jgs:
                    bsl = slice(P * j, P * (j + 1))
                    for g in gl:
                        c0, c1 = GCOLS[g]
                        nc.tensor.matmul(gsls[(j, g)], lhsT=z[0][q][:, bsl],
                                         rhs=R0_sb[:, q, c0:c1],
                                         start=(q == 0), stop=False)
                if prep:
                    # the next replication matmul comes AFTER this q's main
                    # matmuls so a late weight DMA can't block them in-order
                    if q < 6:
                        h_mm(q + 2)
                    else:
                        x_mm(1 if q == 6 else 2)
            if prep:
                x_mm(3)
                for tdx in range(1, 4):
                    for q in range(8):
                        z_mul(tdx, q)
            for j, gl in jgs:
                bsl = slice(P * j, P * (j + 1))
                for g in gl:
                    c0, c1 = GCOLS[g]
                    if not skip_lb2:
                        nc.tensor.matmul(gsls[(j, g)],
                                         lhsT=xbc[0][0:16, bsl],
                                         rhs=R0x_sb[:, c0:c1],
                                         start=False, stop=False)
                    if fp8:
                        nc.tensor.matmul(gsls[(j, g)], lhsT=hbp_sb[:, :, bsl],
                                         rhs=BB_sb[:, :, c0:c1],
                                         start=False, stop=True,
                                         perf_mode=DRM)
                    else:
                        nc.tensor.matmul(gsls[(j, g)], lhsT=hbp_sb[:, bsl],
                                         rhs=BB_sb[:, c0:c1],
                                         start=False, stop=True)
                    copy_out(P1_ENG[j][g], out_t[j][:, c0:c1], gsls[(j, g)])
                nc.sync.dma_start(t["out"][bsl, GCOLS[gl[0]][0]:GCOLS[gl[-1]][1]],
                                  out_t[j][:, GCOLS[gl[0]][0]:GCOLS[gl[-1]][1]])

        phase1_multi([(0, [0, 1, 2])], prep=True)
        phase1_multi([(1, [0, 1, 2])])
        phase1_multi([(2, [0, 1, 2])])


        def p23_group(phase, j, mi):
            bsl = slice(P * j, P * (j + 1))
            rc = (0, 512) if phase == 0 else (512, 1024)
            m = 3 * phase + mi
            tdx = 1 + mi
            pgm = main_psum.tile([P, 512], F32, name=f"pm{j}_{m}", tag="mp")
            if fp8:
                for qq in range(4):
                    nc.tensor.matmul(pgm[:], lhsT=zp[tdx][qq][:, :, bsl],
                                     rhs=R1h_sb[:, phase, qq, :, :],
                                     start=(qq == 0), stop=False,
                                     perf_mode=DRM)
                    nc.tensor.matmul(pgm[:], lhsT=zp[tdx][qq][:, :, bsl],
                                     rhs=R1l_sb[:, phase, qq, :, :],
                                     start=False, stop=(skip_lb2 and qq == 3),
                                     perf_mode=DRM)
            else:
                for q in range(8):
                    last = skip_lb2 and q == 7
                    nc.tensor.matmul(pgm[:], lhsT=z[tdx][q][:, bsl],
                                     rhs=R1_sb[:, q, rc[0]:rc[1]],
                                     start=(q == 0), stop=last)
            if not skip_lb2:
                nc.tensor.matmul(pgm[:], lhsT=xbc[tdx][0:16, bsl],
                                 rhs=R1x_sb[:, rc[0]:rc[1]],
                                 start=False, stop=True)
            c0 = 1280 + 512 * m
            copy_out(P23_ENG[j][m], out_t[j][:, c0:c0 + 512], pgm[:])

        # phase 2 runs mi-major so its first groups only need zp[1][*]
        # (the zp muls for tdx 2/3 are still draining on DVE then);
        # phase 3 stays j-major
        for j, mi in [(j, mi) for mi in range(3) for j in range(NB)]:
            p23_group(0, j, mi)
            if mi == 2:
                bsl = slice(P * j, P * (j + 1))
                nc.sync.dma_start(t["out"][bsl, 1280:2816],
                                  out_t[j][:, 1280:2816])
        for j in range(NB):
            for mi in range(3):
                p23_group(1, j, mi)
            bsl = slice(P * j, P * (j + 1))
            nc.sync.dma_start(t["out"][bsl, 2816:4352],
                              out_t[j][:, 2816:4352])

        phase1(NB - 1, final=True)


def _prepare(inputs, mode):
    f32 = np.float32
    feat = np.ascontiguousarray(np.asarray(inputs["feat"], dtype=f32))
    node_emb = np.ascontiguousarray(np.asarray(inputs["node_emb"], dtype=f32))
    W0 = np.asarray(inputs["W0"], f32)
    W1 = np.asarray(inputs["W1"], f32)
    lw1 = np.asarray(inputs["lw1"], f32)
    lb1 = np.asarray(inputs["lb1"], f32)
    lw2 = np.asarray(inputs["lw2"], f32)
    lb2 = np.asarray(inputs["lb2"], f32)
    bw1 = np.asarray(inputs["bw1"], f32)
    bb1 = np.asarray(inputs["bb1"], f32)
    bw2 = np.asarray(inputs["bw2"], f32)
    bb2 = np.asarray(inputs["bb2"], f32)

    mmnp = _np_mm_dtype(mode)
    s16 = np.float32(1.0 / 16.0)
    sC = np.float32(C3 / 16.0)

    # weight matrices for the main contraction, path scales folded in
    lw2p = np.concatenate([lw2, lb2[None]], axis=0)           # [65, 36864]
    M00 = lw2p[:, :16384].reshape(1040, 1024) * s16
    M11 = lw2p[:, 16384:20480].reshape(1040, 256) * sC
    M01 = lw2p[:, 20480:28672].reshape(1040, 512) * sC
    M10 = lw2p[:, 28672:36864].reshape(1040, 512) * sC
    R0f = np.concatenate([M00, M11], axis=1)                  # [1040, 1280]
    R1f = np.concatenate([M01, M10], axis=1)                  # [1040, 1024]
    fp8 = FP8_P23 and mode == "bf16"
    r0scale = np.float32(R1_SCALE) if fp8 else np.float32(1.0)
    R0 = np.ascontiguousarray(R0f[0:1024] * r0scale).astype(mmnp)
    R0x = np.ascontiguousarray(R0f[1024:1040] * r0scale).astype(mmnp)
    if fp8:
        e4m3 = ml_dtypes.float8_e4m3
        R1s = R1f[0:1024] * np.float32(R1_SCALE)              # [1024, 1024]
        # [qq, s, p, half, n'] -> [p, half, qq, s, n']
        R1q = R1s.reshape(4, 2, P, 2, 512).transpose(2, 3, 0, 1, 4)
        R1hq = R1q.astype(e4m3)
        R1lq = (R1q - R1hq.astype(np.float32)).astype(e4m3)
        R1h = np.ascontiguousarray(R1hq)
        R1l = np.ascontiguousarray(R1lq)
        R1x = np.ascontiguousarray(
            R1f[1024:1040] * np.float32(R1_SCALE)).astype(mmnp)
    else:
        R1 = np.ascontiguousarray(R1f[0:1024]).astype(mmnp)
        R1x = np.ascontiguousarray(R1f[1024:1040]).astype(mmnp)
    BBf = np.concatenate([bw2, bb2[None]], axis=0)            # [65, 1280]
    BBps = np.concatenate([BBf[:, :1024] * s16, BBf[:, 1024:] * sC], axis=1)
    if fp8:
        e4m3 = ml_dtypes.float8_e4m3
        BBs = (BBps * np.float32(R1_SCALE)).astype(np.float32)
        BBhi = BBs.astype(e4m3)
        BBlo = (BBs - BBhi.astype(np.float32)).astype(e4m3)
        BBhl = np.ascontiguousarray(np.stack([BBhi, BBlo], axis=1))
    else:
        BBp = np.ascontiguousarray(BBps).astype(mmnp)

    # replicated-column weights: output partition (c8,w) = 16*c8 + w
    W0s = W0 * np.float32(1.0 / np.sqrt(128.0))               # [128, 16]
    W1s = W1 * np.float32(1.0 / 8.0)                          # [64, 16]
    rep = np.arange(1024)
    gsel = (rep // 128) * 8 + (rep % 128) // 16               # c = 8q + c8
    lw1rep = lw1[:, gsel]                                     # [128, 1024]
    W0rep = np.tile(W0s, (1, 8))                              # [128, 128]
    W1rep = np.tile(W1s, (1, 8))                              # [64, 128]
    # layout must match HCOL & friends in _emit (embT in cols 0:512,
    # filled per core below)
    wpk = np.zeros((128, 1872), f32)
    hcol = [512, 640, 976, 1104, 1232, 1360, 1488, 1616]
    for q in range(8):
        wpk[:, hcol[q]:hcol[q] + 128] = lw1rep[:, 128 * q:128 * (q + 1)]
        wpk[:, 768 + q] = lb1[8 * q + np.arange(128) // 16]
    wpk[0:64, 776] = bb1
    wpk[:, 784:848] = bw1
    wpk[:, 848:976] = W0rep
    wpk[0:64, 1744:1872] = W1rep
    wpk = wpk.astype(mmnp)

    skip_lb2 = not bool(np.any(lb2))

    in_maps = []
    for i in range(N_CORES):
        sl = slice(i * BC, (i + 1) * BC)
        fs = feat[sl]
        featT = np.ascontiguousarray(
            np.concatenate(
                [fs[:, :128], fs[:, 128::3], fs[:, 129::3], fs[:, 130::3]],
                axis=1).T.astype(mmnp))                       # [320, BC]
        wpk_i = wpk.copy()
        wpk_i[:, 0:512] = node_emb[sl].T.astype(mmnp)
        m = {
            "featT": featT,
            "wpk": np.ascontiguousarray(wpk_i),
            "R0": R0,
        }
        if fp8:
            m["BBhl"] = BBhl
            m["R1h"] = R1h
            m["R1l"] = R1l
        else:
            m["BBp"] = BBp
            m["R1"] = R1
        if not skip_lb2:
            m["R0x"] = R0x
            m["R1x"] = R1x
        in_maps.append(m)
    return in_maps, skip_lb2, fp8


def _unpack_output(buf, fp8):
    """[B, 4352] packed columns -> [B, 80, 80] float32.

    In fp8 mode, columns 1280: (blk01/blk10) carry the 2^12 R1 scale;
    descaled here.
    """
    bf = buf.astype(np.float32)
    n = bf.shape[0]
    if fp8:
        bf *= np.float32(R1_DESCALE)
    out3 = np.zeros((n, 80, 80), np.float32)
    out3[:, 0:16, 0:32] = bf[:, 0:512].reshape(n, 16, 32)
    out3[:, 16:32, 0:32] = bf[:, 512:1024].reshape(n, 16, 32)
    p11 = bf[:, 1024:1280].reshape(n, 16, 16)
    for i in range(3):
        out3[:, 32 + i::3, 32 + i::3] = p11
    for k in range(3):
        out3[:, 0:32, 32 + k::3] = \
            bf[:, 1280 + 512 * k:1792 + 512 * k].reshape(n, 32, 16)
    for i in range(3):
        out3[:, 32 + i::3, 0:32] = \
            bf[:, 2816 + 512 * i:3328 + 512 * i].reshape(n, 16, 32)
    return out3


def run(inputs, mode=None, trace=False):
    """Build (cached), run on 8 cores, gather. Returns (out, results)."""
    mode = mode or MM_MODE
    in_maps, skip_lb2, fp8 = _prepare(inputs, mode)
    key = (mode, skip_lb2, fp8)
    if key not in _CACHE:
        _CACHE[key] = _build_program(mode, skip_lb2, fp8)
    nc = _CACHE[key]

    from concourse.bass_utils import run_bass_kernel_spmd
    for attempt in range(3):
        res = run_bass_kernel_spmd(nc, in_maps, list(range(N_CORES)),
                                   trace=trace)
        buf = np.concatenate([res.results[i]["out"] for i in range(N_CORES)],
                             axis=0)
        out = _unpack_output(buf, fp8)
        # guard against a rare transport/device flake observed to return NaN
        # payloads; correct runs are deterministic, so retrying is safe
        if not np.isnan(out).any():
            return out, res
    return out, res


def kernel(**inputs):
    out, _ = run(inputs)
    return out



# revision 56
# speedup vs baseline: 1.4091x; 1.0020x over previous
"""Trainium2 Bass kernel for nn_Expansion (e3nn-style tensor-product expansion).

Math reformulation (verified against the jax reference):
  h   = silu(node_emb @ lw1 + lb1)                         [B,64]
  hb  = silu(node_emb @ bw1 + bb1)                         [B,64]
  x0  = feat[:,:128] @ W0 / sqrt(128)                      [B,16]
  x1k = feat[:,128+k::3] @ W1 / 8          (k=0,1,2)       [B,16]

The per-sample path contractions with wpath = (h @ lw2 + lb2) are a batched
bilinear form

  r[b,p] = sum_{c,w} h'[b,c] x[b,w] M[(c,w), p],   h' = [h, 1]

computed as a plain matmul over the outer product z[b,(c,w)] = h'[b,c]*x[b,w]
(K = 64*16 = 1024 in 8 chunks of 128, + a K=65 chunk for the bias MLP @ BB)
against reshaped weight matrices M built from lw2/bw2 on the host.  This
avoids materializing w = h@lw2 ([B,36864], ~600 MB) entirely.

v2 layout decisions (all driven by the instruction-cost timeline model):
  - The partition-replicated tiles hbc[q][(c8,w),b] = h[8q+c8,b] and
    xbc[t][(c8,w),b] = x_t[w,b] are produced DIRECTLY by the MLP/projection
    matmuls using host-replicated weight columns (lw1rep/W0rep/W1rep), so no
    separate selection-matrix matmuls or extra copies exist.  ACT applies
    silu (with replicated bias) straight from PSUM into bf16 SBUF tiles.
  - All matmuls run in bf16 (fp32 matmuls cost 4x); inputs ship as bf16.
  - The device output is the raw concatenation of the 9 PSUM bank groups per
    sample, [BC, 4352] bf16 — a fixed column permutation of the nonzero
    entries of the [80,80] block matrix.  The host scatters it into the
    final [B,80,80] float32 (incl. the blk11 diagonal triplication and the
    structural zeros), so the device does zero strided/duplicated writes.
  - First b-tile interleaves its three z0-consuming bank groups q-major so
    the PE consumption rate (3 matmuls / z0 chunk) matches ACT's silu
    production rate during warmup.

Sharding: pure data parallel, batch 4096 -> 8 cores x 512.  Weights
replicated; no cross-device communication.

v3: phases 2/3 (blk01/blk10, 69% of the matmul rows) run as fp8e4
DoubleRow matmuls.  Each pair of K-chunks (2q, 2q+1) becomes two
K=256 DoubleRow matmuls -- one against an e4m3 R1_hi pair, one against
the e4m3 residual R1_lo pair -- at 0.5 cycles/row each, halving the
phase-2/3 PE time.  The R side is exact to ~6e-4 (hi+lo split, scaled
x4096 to clear the e4m3 denormal floor; the 2^-12 descale is folded
into the PSUM->SBUF copies).  The z side is a single e4m3 rounding
(sigma ~2.6e-2) applied only to the blk01/blk10 paths (42.6% of output
norm^2), leaving total rel err ~1.8e-2 vs the 2e-2 gate (verified
bit-exact RNE conversion on device).  blk00/blk11 (phase 1) stay bf16.
"""

import sys

import numpy as np

sys.path.insert(0, "/opt/trn_rl_repo")

import ml_dtypes  # noqa: E402

B_TOTAL = 4096
N_CORES = 8
BC = B_TOTAL // N_CORES  # 512 samples per core
P = 128
NB = BC // P  # 4 b-tiles per core
C3 = 1.0 / np.sqrt(3.0)
NCOL = 4352  # packed device output columns per sample

# matmul dtype mode: "bf16" | "f32"
MM_MODE = "bf16"
# fp8e4 DoubleRow for phases 2/3 (blk01/blk10); only in bf16 mode
FP8_P23 = True
R1_SCALE = 4096.0  # lifts e4m3(R1) out of the denormal floor
R1_DESCALE = 1.0 / R1_SCALE
N_WARM = 22   # PE warmup dummies before prep
N_FILL = 6    # PE fillers covering the silu->z_mul latency
PSUM_PREP = 3
PSUM_MAIN = 5

_CACHE = {}


def _np_mm_dtype(mode):
    return ml_dtypes.bfloat16 if mode == "bf16" else np.float32


def _build_program(mode, skip_lb2, fp8):
    import concourse.tile as tile
    from concourse import bacc, mybir

    F32 = mybir.dt.float32
    MM = mybir.dt.bfloat16 if mode == "bf16" else mybir.dt.float32
    AF = mybir.ActivationFunctionType

    nc = bacc.Bacc("TRN2", target_bir_lowering=False, debug=False,
                   num_devices=N_CORES)

    t = {}
    t["featT"] = nc.dram_tensor("featT", [320, BC], MM, kind="ExternalInput").ap()
    t["wpk"] = nc.dram_tensor("wpk", [P, 1872], MM, kind="ExternalInput").ap()
    t["R0"] = nc.dram_tensor("R0", [1024, 1280], MM, kind="ExternalInput").ap()
    if fp8:
        FP8 = mybir.dt.float8e4
        # bias MLP weights as an (hi, e4m3-residual) subtile pair
        t["BBhl"] = nc.dram_tensor("BBhl", [65, 2, 1280], FP8,
                                   kind="ExternalInput").ap()
        # [p, col-half, qq, subtile, n'] -- hi and e4m3 residual pairs
        t["R1h"] = nc.dram_tensor("R1h", [P, 2, 4, 2, 512], FP8,
                                  kind="ExternalInput").ap()
        t["R1l"] = nc.dram_tensor("R1l", [P, 2, 4, 2, 512], FP8,
                                  kind="ExternalInput").ap()
    else:
        t["BBp"] = nc.dram_tensor("BBp", [65, 1280], MM, kind="ExternalInput").ap()
        t["R1"] = nc.dram_tensor("R1", [1024, 1024], MM, kind="ExternalInput").ap()
    if not skip_lb2:
        t["R0x"] = nc.dram_tensor("R0x", [16, 1280], MM, kind="ExternalInput").ap()
        t["R1x"] = nc.dram_tensor("R1x", [16, 1024], MM, kind="ExternalInput").ap()
    t["out"] = nc.dram_tensor("out", [BC, NCOL], MM, kind="ExternalOutput").ap()

    with tile.TileContext(nc) as tc:
        _emit(tc, t, mode, skip_lb2, fp8, mybir, MM, F32, AF)

    nc.compile()
    return nc


def _emit(tc, t, mode, skip_lb2, fp8, mybir, MM, F32, AF):
    nc = tc.nc
    FP8 = mybir.dt.float8e4
    DRM = mybir.MatmulPerfMode.DoubleRow
    from contextlib import ExitStack

    with ExitStack() as ctx:
        wpool = ctx.enter_context(tc.tile_pool(name="weights", bufs=1))
        apool = ctx.enter_context(tc.tile_pool(name="acts", bufs=1))
        zpool = ctx.enter_context(tc.tile_pool(name="z", bufs=1))
        opool = ctx.enter_context(tc.tile_pool(name="outs", bufs=3))
        prep_psum = ctx.enter_context(tc.tile_pool(name="prep_psum", bufs=PSUM_PREP, space="PSUM"))
        main_psum = ctx.enter_context(tc.tile_pool(name="main_psum", bufs=PSUM_MAIN, space="PSUM"))

        # ---- SBUF tiles ----
        # wpk column layout: [embT(512) | lw1rep q0 | q1 | lb1rep(8) |
        #   bb1(1) | pad(7) | bw1(64) | W0rep(128) || lw1rep q2..q7 |
        #   W1rep(128, rows 0:64)]
        # The prefix [0:976] is everything the first prep matmuls need.
        wpk_sb = wpool.tile([P, 1872], MM, tag="wpk")
        R0_sb = wpool.tile([P, 8, 1280], MM, tag="R0")
        if fp8:
            BB_sb = wpool.tile([65, 2, 1280], FP8, tag="BBhl")
            R1h_sb = wpool.tile([P, 2, 4, 2, 512], FP8, tag="R1h")
            R1l_sb = wpool.tile([P, 2, 4, 2, 512], FP8, tag="R1l")
        else:
            BB_sb = wpool.tile([65, 1280], MM, tag="BBp")
            R1_sb = wpool.tile([P, 8, 1024], MM, tag="R1")
        if not skip_lb2:
            R0x_sb = wpool.tile([16, 1280], MM, tag="R0x")
            R1x_sb = wpool.tile([16, 1024], MM, tag="R1x")

        feats_sb = apool.tile([P, BC], MM, tag="feats")
        featv_sb = apool.tile([64, 3, BC], MM, tag="featv")
        if fp8:
            hbp_sb = apool.tile([65, 2, BC], FP8, name="hbp_sb", tag="hbp")
        else:
            hbp_sb = apool.tile([65, BC], MM, name="hbp_sb", tag="hbp")
        hbc = [apool.tile([P, BC], MM, name=f"hbc{q}", tag=f"hbc{q}")
               for q in range(8)]
        xbc = [apool.tile([P, BC], MM, name=f"xbc{t_}", tag=f"xbc{t_}")
               for t_ in range(4)]

        # ---- input DMAs, ordered by first consumer ----
        # R0 arrives q-chunk-major to match the phase-1 q-major consumption;
        # R1 follows in halves (phase 2/3 consume it much later)
        r0v = t["R0"].rearrange("(q p) n -> p q n", p=P)
        nc.sync.dma_start(wpk_sb[:, 0:784], t["wpk"][:, 0:784])
        nc.sync.dma_start(feats_sb[:], t["featT"][0:128])
        nc.sync.dma_start(wpk_sb[:, 784:976], t["wpk"][:, 784:976])
        nc.sync.dma_start(wpk_sb[:, 976:1872], t["wpk"][:, 976:1872])
        nc.sync.dma_start(R0_sb[:, 0, :], r0v[:, 0, :])
        nc.sync.dma_start(R0_sb[:, 1, :], r0v[:, 1, :])
        nc.sync.dma_start(R0_sb[:, 2, :], r0v[:, 2, :])
        nc.sync.dma_start(R0_sb[:, 3, :], r0v[:, 3, :])
        nc.sync.dma_start(R0_sb[:, 4, :], r0v[:, 4, :])
        nc.sync.dma_start(featv_sb[:],
                          t["featT"][128:320].rearrange("(k p) b -> p k b", k=3))
        for q in range(5, 7):
            nc.sync.dma_start(R0_sb[:, q, :], r0v[:, q, :])
        # bias weights land just before their first (end-of-j0) consumer
        nc.sync.dma_start(BB_sb[:], t["BBhl"][:] if fp8 else t["BBp"][:])
        nc.sync.dma_start(R0_sb[:, 7, :], r0v[:, 7, :])
        if fp8:
            nc.sync.dma_start(R1h_sb[:, 0], t["R1h"][:, 0])
            nc.sync.dma_start(R1l_sb[:, 0], t["R1l"][:, 0])
            nc.sync.dma_start(R1h_sb[:, 1], t["R1h"][:, 1])
            nc.sync.dma_start(R1l_sb[:, 1], t["R1l"][:, 1])
        else:
            r1v = t["R1"].rearrange("(q p) n -> p q n", p=P)
            nc.sync.dma_start(R1_sb[:, 0:4, 0:512], r1v[:, 0:4, 0:512])
            nc.sync.dma_start(R1_sb[:, 4:8, 0:512], r1v[:, 4:8, 0:512])
            nc.sync.dma_start(R1_sb[:, 0:4, 512:1024], r1v[:, 0:4, 512:1024])
            nc.sync.dma_start(R1_sb[:, 4:8, 512:1024], r1v[:, 4:8, 512:1024])
        if not skip_lb2:
            nc.sync.dma_start(R0x_sb[:], t["R0x"][:])
            nc.sync.dma_start(R1x_sb[:], t["R1x"][:])

        # ---- PE warmup: dummy matmuls on the framework const APs keep the
        # PE busy (and its p-state ramping) while the first input DMAs land;
        # const APs are valid right after the program-start barrier, ~500ns
        # before any memset this kernel could issue would complete.  Results
        # go to a scratch PSUM bank that is overwritten before any real use.
        cb1 = nc.const_aps.aps[(MM, 1.0)] if mode == "bf16" else             nc.const_aps.aps[(F32, 1.0)]
        warm_lhsT = cb1
        warm_rhs = cb1.to_broadcast([P, P])
        # preload the ACT activation table so the first real silu doesn't pay
        # the ~1.3us table-load latency (input values are irrelevant)
        tbl_sb = apool.tile([1, 4], MM, tag="tbl")
        nc.scalar.activation(tbl_sb[0:1, 0:1], cb1[0:1, 0:1], AF.Silu)
        pwarm = prep_psum.tile([P, P], F32, tag="pp")
        for _ in range(N_WARM):
            nc.tensor.matmul(pwarm[0:1, :], lhsT=warm_lhsT, rhs=warm_rhs,
                             start=True, stop=True)

        # ---- prep emitters ----
        z = [[None] * 8 for _ in range(4)]
        zp = [[None] * 4 for _ in range(4)]  # fp8 chunk-pair tiles, tdx>=1
        HCOL = [512, 640, 976, 1104, 1232, 1360, 1488, 1616]

        def h_mm(q):
            # hbc[q][(c8,w),b] = silu((lw1rep_q)^T emb + lb1rep_q) = h[8q+c8,b]
            phq = prep_psum.tile([P, BC], F32, name=f"ph{q}", tag="pp")
            nc.tensor.matmul(phq[:], lhsT=wpk_sb[:, HCOL[q]:HCOL[q] + P],
                             rhs=wpk_sb[:, 0:512], start=True, stop=True)
            nc.scalar.activation(hbc[q][:], phq[:], AF.Silu,
                                 bias=wpk_sb[:, 768 + q:769 + q])

        def x_mm(tdx):
            # xbc[t][(c8,w),b] = x_t[w,b] via column-replicated W0/W1
            # (copy on DVE — ACT is saturated by the silu chain during prep)
            pxt = prep_psum.tile([P, BC], F32, name=f"px{tdx}", tag="pp")
            if tdx == 0:
                nc.tensor.matmul(pxt[:], lhsT=wpk_sb[:, 848:976],
                                 rhs=feats_sb[:], start=True, stop=True)
            else:
                nc.tensor.matmul(pxt[:], lhsT=wpk_sb[0:64, 1744:1872],
                                 rhs=featv_sb[:, tdx - 1, :],
                                 start=True, stop=True)
            # ACT copy: DVE is the zp-mul critical path into phase 2
            nc.scalar.copy(xbc[tdx][:], pxt[:])

        def z_mul(tdx, q):
            if fp8 and tdx >= 1:
                # e4m3 z, written into subtile slot q%2 of the (2q', 2q'+1)
                # DoubleRow pair tile (single RNE rounding off the bf16 mul)
                qq = q // 2
                if zp[tdx][qq] is None:
                    zp[tdx][qq] = zpool.tile([P, 2, BC], FP8,
                                             name=f"zp{tdx}_{qq}",
                                             tag=f"zp{tdx}_{qq}")
                nc.vector.tensor_mul(out=zp[tdx][qq][:, q % 2, :],
                                     in0=hbc[q][:], in1=xbc[tdx][:])
                return
            zt = zpool.tile([P, BC], MM, name=f"z{tdx}_{q}", tag=f"z{tdx}_{q}")
            nc.vector.tensor_mul(out=zt[:], in0=hbc[q][:], in1=xbc[tdx][:])
            z[tdx][q] = zt

        h_mm(0)
        x_mm(0)
        h_mm(1)

        # hbp[c,b] = silu(bw1^T emb + bb1), plus a ones row for the bb2
        # path.  fp8 mode: e4m3 pair tile, the same values in both DoubleRow
        # subtiles (paired against the host-built BB hi/lo rhs).
        ph = prep_psum.tile([64, BC], F32, tag="pp")
        nc.tensor.matmul(ph[:], lhsT=wpk_sb[:, 784:848], rhs=wpk_sb[:, 0:512],
                         start=True, stop=True)
        if fp8:
            nc.scalar.activation(hbp_sb[0:64, 0, :], ph[:], AF.Silu,
                                 bias=wpk_sb[0:64, 776:777])
            nc.vector.tensor_copy(out=hbp_sb[0:64, 1, :],
                                  in_=hbp_sb[0:64, 0, :])
            nc.gpsimd.memset(hbp_sb[64:65, :, :], 1.0)
        else:
            nc.scalar.activation(hbp_sb[0:64, :], ph[:], AF.Silu,
                                 bias=wpk_sb[0:64, 776:777])
            nc.gpsimd.memset(hbp_sb[64:65, :], 1.0)

        # fillers: cover the silu->z_mul latency before the first z matmul
        for _ in range(N_FILL):
            nc.tensor.matmul(pwarm[0:1, :], lhsT=warm_lhsT, rhs=warm_rhs,
                             start=True, stop=True)

        # ---- main accumulation groups, group-type-major ----
        # phase 1 (needs only R0): per b-tile g0..g2 = z0 @ R0 cols
        # (0:512 | 512:1024 | 1024:1280) + the BB bias chunk, q-major
        # interleaved across the three banks.  The j==0 pass interleaves the
        # remaining prep matmuls so the PE tracks ACT's silu cadence.
        # phase 2 (R1[:, 0:512]): g3..g5 = z[1+k] @ R1 left  (blk01)
        # phase 3 (R1[:, 512:1024]): g6..g8 = z[1+i] @ R1 right (blk10)
        def copy_out(eng, dst_ap, src_ap):
            # blk01/blk10 columns keep the 2^12 R1 scale in bf16 (relative
            # precision is scale-free); the host descales during unpack
            if eng == "a":
                nc.scalar.copy(dst_ap, src_ap)
            elif eng == "p":
                nc.gpsimd.tensor_copy(out=dst_ap, in_=src_ap)
            else:
                nc.vector.tensor_copy(out=dst_ap, in_=src_ap)

        out_t = [opool.tile([P, NCOL], MM, name=f"out_t{j}", tag=f"out_t{j}")
                 for j in range(NB)]
        P1_ENG = ['aaa', 'aaa', 'aaa', 'add']  # per-j engines for g0/g1/g2
        P23_ENG = ['adadad', 'dadada', 'adadad', 'dadada']  # per-j g3..g8

        def phase1(j, final):
            bsl = slice(P * j, P * (j + 1))
            gcols = [(0, 512), (512, 1024), (1024, 1280)]
            if final:
                # sequential groups with per-group writeback: only the last
                # (smallest) group's copy+DMA trail the final matmul
                for g, (c0, c1) in enumerate(gcols):
                    pg = main_psum.tile([P, 512], F32, name=f"pg{j}_{g}",
                                        tag="mp")
                    psl = pg[:, 0:c1 - c0]
                    for q in range(8):
                        nc.tensor.matmul(psl, lhsT=z[0][q][:, bsl],
                                         rhs=R0_sb[:, q, c0:c1],
                                         start=(q == 0), stop=False)
                    if not skip_lb2:
                        nc.tensor.matmul(psl, lhsT=xbc[0][0:16, bsl],
                                         rhs=R0x_sb[:, c0:c1],
                                         start=False, stop=False)
                    if fp8:
                        nc.tensor.matmul(psl, lhsT=hbp_sb[:, :, bsl],
                                         rhs=BB_sb[:, :, c0:c1],
                                         start=False, stop=True,
                                         perf_mode=DRM)
                    else:
                        nc.tensor.matmul(psl, lhsT=hbp_sb[:, bsl],
                                         rhs=BB_sb[:, c0:c1],
                                         start=False, stop=True)
                    # alternate copy engines and DMA queues so the
                    # trailing writebacks generate descriptors in parallel
                    copy_out("ada"[g], out_t[j][:, c0:c1], psl)
                    dma_eng = [nc.scalar, nc.gpsimd, nc.sync][g]
                    dma_eng.dma_start(t["out"][bsl, c0:c1],
                                      out_t[j][:, c0:c1])
                return
            raise AssertionError("non-final phase1 goes through phase1_multi")

        GCOLS = [(0, 512), (512, 1024), (1024, 1280)]

        def phase1_multi(jgs, prep=False):
            # q-major across (b-tile, group-subset) pairs: every R0 chunk is
            # consumed by all listed groups right as its DMA lands, so later
            # b-tiles don't sit behind j0's R0-starved matmuls in the
            # in-order PE stream.  Partial group lists keep the in-flight
            # PSUM banks within the pool.  j0's prep interleave rides along.
            gsls = {}
            for j, gl in jgs:
                for g in gl:
                    c0, c1 = GCOLS[g]
                    pg = main_psum.tile([P, 512], F32, name=f"pg{j}_{g}",
                                        tag="mp")
                    gsls[(j, g)] = pg[:, 0:c1 - c0]
            if prep:
                z_mul(0, 0)
            for q in range(8):
                if prep and q < 7:
                    # produce z0[q+1] one slot ahead of its consumers so the
                    # silu->mul latency never stalls the main matmul stream
                    z_mul(0, q + 1)
                for j, gl in jgs:
                    bsl = slice(P * j, P * (j + 1))
                    for g in gl:
                        c0, c1 = GCOLS[g]
                        nc.tensor.matmul(gsls[(j, g)], lhsT=z[0][q][:, bsl],
                                         rhs=R0_sb[:, q, c0:c1],
                                         start=(q == 0), stop=False)
                if prep:
                    # the next replication matmul comes AFTER this q's main
                    # matmuls so a late weight DMA can't block them in-order
                    if q < 6:
                        h_mm(q + 2)
                    else:
                        x_mm(1 if q == 6 else 2)
            if prep:
                x_mm(3)
                for tdx in range(1, 4):
                    for q in range(8):
                        z_mul(tdx, q)
            for j, gl in jgs:
                bsl = slice(P * j, P * (j + 1))
                for g in gl:
                    c0, c1 = GCOLS[g]
                    if not skip_lb2:
                        nc.tensor.matmul(gsls[(j, g)],
                                         lhsT=xbc[0][0:16, bsl],
                                         rhs=R0x_sb[:, c0:c1],
                                         start=False, stop=False)
                    if fp8:
                        nc.tensor.matmul(gsls[(j, g)], lhsT=hbp_sb[:, :, bsl],
                                         rhs=BB_sb[:, :, c0:c1],
                                         start=False, stop=True,
                                         perf_mode=DRM)
                    else:
                        nc.tensor.matmul(gsls[(j, g)], lhsT=hbp_sb[:, bsl],
                                         rhs=BB_sb[:, c0:c1],
                                         start=False, stop=True)
                    copy_out(P1_ENG[j][g], out_t[j][:, c0:c1], gsls[(j, g)])
                nc.sync.dma_start(t["out"][bsl, GCOLS[gl[0]][0]:GCOLS[gl[-1]][1]],
                                  out_t[j][:, GCOLS[gl[0]][0]:GCOLS[gl[-1]][1]])

        phase1_multi([(0, [0, 1, 2])], prep=True)
        phase1_multi([(1, [0, 1, 2])])
        phase1_multi([(2, [0, 1, 2])])


        def p23_group(phase, j, mi):
            bsl = slice(P * j, P * (j + 1))
            rc = (0, 512) if phase == 0 else (512, 1024)
            m = 3 * phase + mi
            tdx = 1 + mi
            pgm = main_psum.tile([P, 512], F32, name=f"pm{j}_{m}", tag="mp")
            if fp8:
                for qq in range(4):
                    nc.tensor.matmul(pgm[:], lhsT=zp[tdx][qq][:, :, bsl],
                                     rhs=R1h_sb[:, phase, qq, :, :],
                                     start=(qq == 0), stop=False,
                                     perf_mode=DRM)
                    nc.tensor.matmul(pgm[:], lhsT=zp[tdx][qq][:, :, bsl],
                                     rhs=R1l_sb[:, phase, qq, :, :],
                                     start=False, stop=(skip_lb2 and qq == 3),
                                     perf_mode=DRM)
            else:
                for q in range(8):
                    last = skip_lb2 and q == 7
                    nc.tensor.matmul(pgm[:], lhsT=z[tdx][q][:, bsl],
                                     rhs=R1_sb[:, q, rc[0]:rc[1]],
                                     start=(q == 0), stop=last)
            if not skip_lb2:
                nc.tensor.matmul(pgm[:], lhsT=xbc[tdx][0:16, bsl],
                                 rhs=R1x_sb[:, rc[0]:rc[1]],
                                 start=False, stop=True)
            c0 = 1280 + 512 * m
            copy_out(P23_ENG[j][m], out_t[j][:, c0:c0 + 512], pgm[:])

        # phase 2 runs mi-major so its first groups only need zp[1][*]
        # (the zp muls for tdx 2/3 are still draining on DVE then);
        # phase 3 stays j-major
        for j, mi in [(j, mi) for mi in range(3) for j in range(NB)]:
            p23_group(0, j, mi)
            if mi == 2:
                bsl = slice(P * j, P * (j + 1))
                nc.sync.dma_start(t["out"][bsl, 1280:2816],
                                  out_t[j][:, 1280:2816])
        for j in range(NB):
            for mi in range(3):
                p23_group(1, j, mi)
            bsl = slice(P * j, P * (j + 1))
            nc.sync.dma_start(t["out"][bsl, 2816:4352],
                              out_t[j][:, 2816:4352])

        phase1(NB - 1, final=True)


def _prepare(inputs, mode):
    f32 = np.float32
    feat = np.ascontiguousarray(np.asarray(inputs["feat"], dtype=f32))
    node_emb = np.ascontiguousarray(np.asarray(inputs["node_emb"], dtype=f32))
    W0 = np.asarray(inputs["W0"], f32)
    W1 = np.asarray(inputs["W1"], f32)
    lw1 = np.asarray(inputs["lw1"], f32)
    lb1 = np.asarray(inputs["lb1"], f32)
    lw2 = np.asarray(inputs["lw2"], f32)
    lb2 = np.asarray(inputs["lb2"], f32)
    bw1 = np.asarray(inputs["bw1"], f32)
    bb1 = np.asarray(inputs["bb1"], f32)
    bw2 = np.asarray(inputs["bw2"], f32)
    bb2 = np.asarray(inputs["bb2"], f32)

    mmnp = _np_mm_dtype(mode)
    s16 = np.float32(1.0 / 16.0)
    sC = np.float32(C3 / 16.0)

    # weight matrices for the main contraction, path scales folded in
    lw2p = np.concatenate([lw2, lb2[None]], axis=0)           # [65, 36864]
    M00 = lw2p[:, :16384].reshape(1040, 1024) * s16
    M11 = lw2p[:, 16384:20480].reshape(1040, 256) * sC
    M01 = lw2p[:, 20480:28672].reshape(1040, 512) * sC
    M10 = lw2p[:, 28672:36864].reshape(1040, 512) * sC
    R0f = np.concatenate([M00, M11], axis=1)                  # [1040, 1280]
    R1f = np.concatenate([M01, M10], axis=1)                  # [1040, 1024]
    fp8 = FP8_P23 and mode == "bf16"
    r0scale = np.float32(R1_SCALE) if fp8 else np.float32(1.0)
    R0 = np.ascontiguousarray(R0f[0:1024] * r0scale).astype(mmnp)
    R0x = np.ascontiguousarray(R0f[1024:1040] * r0scale).astype(mmnp)
    if fp8:
        e4m3 = ml_dtypes.float8_e4m3
        R1s = R1f[0:1024] * np.float32(R1_SCALE)              # [1024, 1024]
        # [qq, s, p, half, n'] -> [p, half, qq, s, n']
        R1q = R1s.reshape(4, 2, P, 2, 512).transpose(2, 3, 0, 1, 4)
        R1hq = R1q.astype(e4m3)
        R1lq = (R1q - R1hq.astype(np.float32)).astype(e4m3)
        R1h = np.ascontiguousarray(R1hq)
        R1l = np.ascontiguousarray(R1lq)
        R1x = np.ascontiguousarray(
            R1f[1024:1040] * np.float32(R1_SCALE)).astype(mmnp)
    else:
        R1 = np.ascontiguousarray(R1f[0:1024]).astype(mmnp)
        R1x = np.ascontiguousarray(R1f[1024:1040]).astype(mmnp)
    BBf = np.concatenate([bw2, bb2[None]], axis=0)            # [65, 1280]
    BBps = np.concatenate([BBf[:, :1024] * s16, BBf[:, 1024:] * sC], axis=1)
    if fp8:
        e4m3 = ml_dtypes.float8_e4m3
        BBs = (BBps * np.float32(R1_SCALE)).astype(np.float32)
        BBhi = BBs.astype(e4m3)
        BBlo = (BBs - BBhi.astype(np.float32)).astype(e4m3)
        BBhl = np.ascontiguousarray(np.stack([BBhi, BBlo], axis=1))
    else:
        BBp = np.ascontiguousarray(BBps).astype(mmnp)

    # replicated-column weights: output partition (c8,w) = 16*c8 + w
    W0s = W0 * np.float32(1.0 / np.sqrt(128.0))               # [128, 16]
    W1s = W1 * np.float32(1.0 / 8.0)                          # [64, 16]
    rep = np.arange(1024)
    gsel = (rep // 128) * 8 + (rep % 128) // 16               # c = 8q + c8
    lw1rep = lw1[:, gsel]                                     # [128, 1024]
    W0rep = np.tile(W0s, (1, 8))                              # [128, 128]
    W1rep = np.tile(W1s, (1, 8))                              # [64, 128]
    # layout must match HCOL & friends in _emit (embT in cols 0:512,
    # filled per core below)
    wpk = np.zeros((128, 1872), f32)
    hcol = [512, 640, 976, 1104, 1232, 1360, 1488, 1616]
    for q in range(8):
        wpk[:, hcol[q]:hcol[q] + 128] = lw1rep[:, 128 * q:128 * (q + 1)]
        wpk[:, 768 + q] = lb1[8 * q + np.arange(128) // 16]
    wpk[0:64, 776] = bb1
    wpk[:, 784:848] = bw1
    wpk[:, 848:976] = W0rep
    wpk[0:64, 1744:1872] = W1rep
    wpk = wpk.astype(mmnp)

    skip_lb2 = not bool(np.any(lb2))

    in_maps = []
    for i in range(N_CORES):
        sl = slice(i * BC, (i + 1) * BC)
        fs = feat[sl]
        featT = np.ascontiguousarray(
            np.concatenate(
                [fs[:, :128], fs[:, 128::3], fs[:, 129::3], fs[:, 130::3]],
                axis=1).T.astype(mmnp))                       # [320, BC]
        wpk_i = wpk.copy()
        wpk_i[:, 0:512] = node_emb[sl].T.astype(mmnp)
        m = {
            "featT": featT,
            "wpk": np.ascontiguousarray(wpk_i),
            "R0": R0,
        }
        if fp8:
            m["BBhl"] = BBhl
            m["R1h"] = R1h
            m["R1l"] = R1l
        else:
            m["BBp"] = BBp
            m["R1"] = R1
        if not skip_lb2:
            m["R0x"] = R0x
            m["R1x"] = R1x
        in_maps.append(m)
    return in_maps, skip_lb2, fp8


def _unpack_output(buf, fp8):
    """[B, 4352] packed columns -> [B, 80, 80] float32.

    In fp8 mode, columns 1280: (blk01/blk10) carry the 2^12 R1 scale;
    descaled here.
    """
    bf = buf.astype(np.float32)
    n = bf.shape[0]
    if fp8:
        bf *= np.float32(R1_DESCALE)
    out3 = np.zeros((n, 80, 80), np.float32)
    out3[:, 0:16, 0:32] = bf[:, 0:512].reshape(n, 16, 32)
    out3[:, 16:32, 0:32] = bf[:, 512:1024].reshape(n, 16, 32)
    p11 = bf[:, 1024:1280].reshape(n, 16, 16)
    for i in range(3):
        out3[:, 32 + i::3, 32 + i::3] = p11
    for k in range(3):
        out3[:, 0:32, 32 + k::3] = \
            bf[:, 1280 + 512 * k:1792 + 512 * k].reshape(n, 32, 16)
    for i in range(3):
        out3[:, 32 + i::3, 0:32] = \
            bf[:, 2816 + 512 * i:3328 + 512 * i].reshape(n, 16, 32)
    return out3


def run(inputs, mode=None, trace=False):
    """Build (cached), run on 8 cores, gather. Returns (out, results)."""
    mode = mode or MM_MODE
    in_maps, skip_lb2, fp8 = _prepare(inputs, mode)
    key = (mode, skip_lb2, fp8)
    if key not in _CACHE:
        _CACHE[key] = _build_program(mode, skip_lb2, fp8)
    nc = _CACHE[key]

    from concourse.bass_utils import run_bass_kernel_spmd
    for attempt in range(3):
        res = run_bass_kernel_spmd(nc, in_maps, list(range(N_CORES)),
                                   trace=trace)
        buf = np.concatenate([res.results[i]["out"] for i in range(N_CORES)],
                             axis=0)
        out = _unpack_output(buf, fp8)
        # guard against a rare transport/device flake observed to return NaN
        # payloads; correct runs are deterministic, so retrying is safe
        if not np.isnan(out).any():
            return out, res
    return out, res


def kernel(**inputs):
    out, _ = run(inputs)
    return out

